# revision 6
# baseline (speedup 1.0000x reference)
"""Trainium2 Bass kernel for nn_Model_40827959116312 (GIN message passing + MLP head).

Self-contained: builds per-core graph structures on host (numpy), compiles a
Bass/Tile SPMD program for 8 NeuronCores, runs via run_bass_kernel_spmd, and
returns the full [64, 10] output.

Sharding: data-parallel over destination nodes (1250 per core, 10 blocks of
128). Layer-1 aggregation runs on host-pregathered x rows (the projection
commutes with the mean-aggregation, so only the 21-wide augmented features are
aggregated). Layer-2 gathers bf16 h1 rows via dma_gather with per-block
source deduplication; descriptor generation is pre-staged (prepare_only +
trigger_dma) so the Q7 descgen overlaps layer 1 and the collectives.
Segment-sums are one-hot/count matmuls accumulating in PSUM. BatchNorm
statistics are [2,512] AllReduces; h1 is AllGathered (bf16) for the layer-2
gather. The MLP head is replicated on every core (feature-major bf16, fused
BN+ReLU activations)."""

import os
import numpy as np
import ml_dtypes

bf16 = ml_dtypes.bfloat16

# Problem constants (from spec).
N, E, B, KCAND = 10000, 160000, 64, 10
DIN, D, DH = 20, 512, 256
NCORES = 8
NLOC = N // NCORES            # 1250
NBLK = (NLOC + 127) // 128    # 10
NPAD = NBLK * 128             # 1280
XP = 32                       # padded x feature width (host-pregathered)
KAUG = DIN + 1                # 21 (features + bias row)
BN_EPS = 1e-5
SUB = 8                       # gather sub-call size: SUB*128 = 1024 indices
NPRE = 4                      # gather sub-calls prepared before layer 1
GH_BUFS = 5                   # SBUF buffers for gather destinations

LAST_EXEC_NS = None           # set by kernel() when profiling succeeds


# ---------------------------------------------------------------------------
# Host-side preprocessing
# ---------------------------------------------------------------------------

def preprocess(x, pg_emb, neigh_emb, W_init, b_init, g1, be1, g2, be2,
               W_fc, b_fc, W_fc2, b_fc2, W_fc3, b_fc3, W_fc4, b_fc4,
               gb, bb, gb2, bb2, gb3, bb3, edge_src, edge_dst, node2graph):
    """Build per-core input maps + the uniform per-block tile count T_BLK."""
    x = np.asarray(x, np.float32)
    edge_src = np.asarray(edge_src, np.int64)
    edge_dst = np.asarray(edge_dst, np.int64)
    node2graph = np.asarray(node2graph, np.int64)

    deg = np.bincount(edge_dst, minlength=N).astype(np.float64)
    invdeg = (1.0 / np.maximum(deg, 1.0)).astype(np.float32)
    r = (deg > 0).astype(np.float32)

    # Per (core, block): unique sources + count matrices.
    per_core = []
    t_blk = SUB
    for c in range(NCORES):
        lo = c * NLOC
        sel = (edge_dst >= lo) & (edge_dst < lo + NLOC)
        s_c = edge_src[sel]
        d_c = edge_dst[sel] - lo
        blocks = []
        for bidx in range(NBLK):
            bsel = (d_c >> 7) == bidx
            s_b = s_c[bsel]
            d_b = d_c[bsel] - (bidx << 7)
            uniq, inv = np.unique(s_b, return_inverse=True)
            blocks.append((uniq, inv, d_b))
            t_blk = max(t_blk, (len(uniq) + 127) // 128)
        per_core.append(blocks)

    T_BLK = ((t_blk + SUB - 1) // SUB) * SUB   # multiple of SUB
    NT = NBLK * T_BLK              # total edge tiles per core
    NU = T_BLK * 128               # padded unique srcs per block

    xpad = np.zeros((N, XP), np.float32)
    xpad[:, :DIN] = x
    xpad_bf = xpad.astype(bf16)

    W_aug = np.zeros((KAUG, D), np.float32)
    W_aug[:DIN] = np.asarray(W_init, np.float32)
    W_aug[DIN] = np.asarray(b_init, np.float32)

    cnt = np.bincount(node2graph, minlength=B).astype(np.float64)
    inv_cnt = (1.0 / np.maximum(cnt, 1.0)).astype(np.float32).reshape(B, 1)

    # Head weights, feature-major bf16 layouts.
    W_fc = np.asarray(W_fc, np.float32)      # [1536, 256]
    Wfc1t = np.ascontiguousarray(
        W_fc.reshape(12, 128, DH).transpose(1, 0, 2)).astype(bf16)
    Wfc2t = np.ascontiguousarray(
        np.asarray(W_fc2, np.float32).reshape(2, 128, DH).transpose(1, 0, 2)).astype(bf16)
    Wfc3t = np.ascontiguousarray(
        np.asarray(W_fc3, np.float32).reshape(2, 128, DH).transpose(1, 0, 2)).astype(bf16)
    Wfc4t = np.ascontiguousarray(
        np.asarray(W_fc4, np.float32).reshape(2, 128, 1).transpose(1, 0, 2)).astype(bf16)

    def ppart(v):  # [256] -> [128, 2] (dh = kt*128 + p)
        return np.ascontiguousarray(np.asarray(v, np.float32).reshape(2, 128).T)

    pgT = np.ascontiguousarray(
        np.asarray(pg_emb, np.float32).T.reshape(4, 128, B).transpose(1, 0, 2)).astype(bf16)
    neighT = np.ascontiguousarray(
        np.asarray(neigh_emb, np.float32).reshape(B * KCAND, D).T
        .reshape(4, 128, B * KCAND).transpose(1, 0, 2)).astype(bf16)
    b4rep = np.full((128, 1), float(np.asarray(b_fc4).reshape(-1)[0]), np.float32)

    shared = dict(
        W_aug=W_aug,
        bn1g=np.asarray(g1, np.float32).reshape(1, D),
        bn1b=np.asarray(be1, np.float32).reshape(1, D),
        bn2g=np.asarray(g2, np.float32).reshape(1, D),
        bn2b=np.asarray(be2, np.float32).reshape(1, D),
        invcnt=inv_cnt,
        Wfc1t=Wfc1t, Wfc2t=Wfc2t, Wfc3t=Wfc3t, Wfc4t=Wfc4t,
        gbT=ppart(gb), bbT=ppart(bb),
        gb2T=ppart(gb2), bb2T=ppart(bb2),
        gb3T=ppart(gb3), bb3T=ppart(bb3),
        b4rep=b4rep,
        pgT=pgT, neighT=neighT,
    )

    in_maps = []
    for c in range(NCORES):
        lo = c * NLOC
        S = np.zeros((NT, 128, 128), np.float32)
        idx_flat = np.zeros(NT * 128, np.int64)
        for bidx in range(NBLK):
            uniq, inv, d_b = per_core[c][bidx]
            Sb = np.zeros((NU, 128), np.float32)
            np.add.at(Sb, (inv, d_b), 1.0)
            S[bidx * T_BLK:(bidx + 1) * T_BLK] = Sb.reshape(T_BLK, 128, 128)
            idx_flat[bidx * NU: bidx * NU + len(uniq)] = uniq
        # wrap: slot i lives at [i % 16, i // 16], tiled over 128 partitions
        idx_w = np.tile(idx_flat.reshape(-1, 16).T, (8, 1)).astype(np.int16)
        # host-pregathered x rows in the device SBUF layout [128, NT, XP]
        Gx = np.ascontiguousarray(xpad_bf[idx_flat.reshape(NT, 128).T])

        nloc_ids = np.arange(NPAD)
        real = nloc_ids < NLOC
        gids = np.minimum(lo + nloc_ids, N - 1)

        xTaug = np.zeros((KAUG, NPAD), np.float32)
        xTaug[:DIN, :NLOC] = x[lo:lo + NLOC].T
        xTaug[DIN, :NLOC] = 1.0 + r[lo:lo + NLOC]

        invdegb = np.zeros((1, NPAD), np.float32)
        invdegb[0, :NLOC] = invdeg[lo:lo + NLOC]

        invdeg_pp = np.where(real, invdeg[gids], 0.0).reshape(NBLK, 128).T.copy()
        maskn = real.astype(np.float32).reshape(NBLK, 128).T.copy()

        Cro = np.zeros((NBLK, 128, B), np.float32)
        n2g_loc = node2graph[lo:lo + NLOC]
        Cro.reshape(NPAD, B)[nloc_ids[real], n2g_loc] = 1.0

        m = dict(shared)
        m.update(
            idx=idx_w,
            S=S.astype(bf16),
            Gx=Gx,
            xTaug=xTaug,
            invdegb=invdegb,
            invdeg_pp=invdeg_pp,
            maskn=maskn,
            Cro=Cro,
        )
        in_maps.append(m)

    return in_maps, T_BLK


# ---------------------------------------------------------------------------
# Device program
# ---------------------------------------------------------------------------

def build_nc(T_BLK, stage=99):
    from contextlib import ExitStack

    import concourse.bass as bass
    import concourse.mybir as mybir
    import concourse.tile as tile
    from concourse import bacc
    from concourse.bass import ts
    from concourse.masks import make_identity

    f32 = mybir.dt.float32
    bf = mybir.dt.bfloat16
    i16 = mybir.dt.int16
    AF = mybir.ActivationFunctionType
    ALU = mybir.AluOpType

    NT = NBLK * T_BLK
    NCALL = NT // SUB              # layer-2 gather sub-calls
    CPB = T_BLK // SUB             # sub-calls per block
    RG = [list(range(NCORES))]

    class _StageDone(Exception):
        pass

    nc = bacc.Bacc("TRN2", target_bir_lowering=False, debug=False,
                   num_devices=NCORES)

    def din(name, shape, dt):
        return nc.dram_tensor(name, list(shape), dt, kind="ExternalInput").ap()

    idx = din("idx", (128, NT * 8), i16)
    S = din("S", (NT, 128, 128), bf)
    Gx_d = din("Gx", (128, NT, XP), bf)
    xTaug = din("xTaug", (KAUG, NPAD), f32)
    invdegb = din("invdegb", (1, NPAD), f32)
    invdeg_pp = din("invdeg_pp", (128, NBLK), f32)
    maskn = din("maskn", (128, NBLK), f32)
    W_aug = din("W_aug", (KAUG, D), f32)
    bn1g = din("bn1g", (1, D), f32)
    bn1b = din("bn1b", (1, D), f32)
    bn2g = din("bn2g", (1, D), f32)
    bn2b = din("bn2b", (1, D), f32)
    Cro = din("Cro", (NBLK, 128, B), f32)
    invcnt = din("invcnt", (B, 1), f32)
    Wfc1t = din("Wfc1t", (128, 12, DH), bf)
    Wfc2t = din("Wfc2t", (128, 2, DH), bf)
    Wfc3t = din("Wfc3t", (128, 2, DH), bf)
    Wfc4t = din("Wfc4t", (128, 2, 1), bf)
    gbT = din("gbT", (128, 2), f32)
    bbT = din("bbT", (128, 2), f32)
    gb2T = din("gb2T", (128, 2), f32)
    bb2T = din("bb2T", (128, 2), f32)
    gb3T = din("gb3T", (128, 2), f32)
    bb3T = din("bb3T", (128, 2), f32)
    b4rep = din("b4rep", (128, 1), f32)
    pgT = din("pgT", (128, 4, B), bf)
    neighT = din("neighT", (128, 4, B * KCAND), bf)

    outp = nc.dram_tensor("outp", [B, KCAND], f32, kind="ExternalOutput").ap()

    with tile.TileContext(nc) as tc, ExitStack() as ctx:
     try:
        const = ctx.enter_context(tc.tile_pool(name="const", bufs=1))
        dram = ctx.enter_context(tc.tile_pool(name="dram", bufs=1, space="DRAM"))
        # PSUM static budget (8 banks): work(2) + sum(1) + ssq(1) + hm(4).
        psA = ctx.enter_context(tc.tile_pool(name="psA", bufs=2, space="PSUM"))
        psStat = ctx.enter_context(tc.tile_pool(name="psStat", bufs=1, space="PSUM"))
        psH = ctx.enter_context(tc.tile_pool(name="psH", bufs=1, space="PSUM"))
        vp = ctx.enter_context(tc.tile_pool(name="vp", bufs=1))
        sq_pool = ctx.enter_context(tc.tile_pool(name="sq", bufs=2))
        keep = ctx.enter_context(tc.tile_pool(name="keep", bufs=1))

        # ---- collective warmup (absorbs first-collective setup + skew) ----
        wu_in = dram.tile([1, 16], f32)
        wu_out = dram.tile([1, 16], f32)
        wu_sb = const.tile([1, 16], f32)
        nc.vector.memset(wu_sb, 1.0)
        nc.sync.dma_start(out=wu_in[:], in_=wu_sb)
        nc.gpsimd.collective_compute(
            "AllReduce", ALU.add, replica_groups=RG,
            ins=[wu_in.opt()], outs=[wu_out.opt()])

        # ---- resident loads ----
        sctx = ctx.enter_context(ExitStack())
        spool = sctx.enter_context(tc.tile_pool(name="spool", bufs=1, side="right"))
        idx_sb = spool.tile([128, NT * 8], i16)
        nc.sync.dma_start(out=idx_sb, in_=idx)
        S_sb = spool.tile([128, NT, 128], bf)
        nc.sync.dma_start(out=S_sb, in_=S.rearrange("t e d -> e t d"))
        Waug_sb = const.tile([KAUG, D], f32)
        nc.sync.dma_start(out=Waug_sb, in_=W_aug)
        invpp_sb = const.tile([128, NBLK], f32)
        nc.sync.dma_start(out=invpp_sb, in_=invdeg_pp)
        mask_sb = const.tile([128, NBLK], f32)
        nc.sync.dma_start(out=mask_sb, in_=maskn)
        g1_sb = const.tile([1, D], f32)
        nc.sync.dma_start(out=g1_sb, in_=bn1g)
        be1_sb = const.tile([1, D], f32)
        nc.sync.dma_start(out=be1_sb, in_=bn1b)
        g2_sb = const.tile([1, D], f32)
        nc.sync.dma_start(out=g2_sb, in_=bn2g)
        be2_sb = const.tile([1, D], f32)
        nc.sync.dma_start(out=be2_sb, in_=bn2b)
        C_sb = const.tile([128, NBLK, B], f32)
        nc.sync.dma_start(out=C_sb, in_=Cro.rearrange("b p g -> p b g"))
        invcnt_sb = const.tile([B, 1], f32)
        nc.sync.dma_start(out=invcnt_sb, in_=invcnt)

        ones1 = const.tile([1, 128], f32)
        nc.vector.memset(ones1, 1.0)
        eps1 = const.tile([1, 1], f32)
        nc.vector.memset(eps1, BN_EPS)
        eps128 = const.tile([128, 1], f32)
        nc.vector.memset(eps128, BN_EPS)

        dbg_done = []

        def dbg_out(src_ap, dt_src, p=64):
            dbs = const.tile([64, 10], f32, name="dbg")
            nc.vector.memset(dbs, 0.0)
            nc.vector.tensor_copy(dbs[0:p, :], src_ap)
            nc.sync.dma_start(out=outp, in_=dbs)
            dbg_done.append(True)

        h1_sb = keep.tile([128, NBLK, D], bf)      # bf16 h1 (local rows)
        h2_sb = keep.tile([128, NBLK, D], f32)     # fp32 h2 (local rows)

        h1loc = dram.tile([NLOC, D], bf)
        h1full = dram.tile([N, D], bf, addr_space="Shared")
        bnc_in = [dram.tile([1, 2 * D], f32, name=f"bi{i}") for i in range(2)]
        bnc_out = [dram.tile([1, 2 * D], f32, name=f"bo{i}") for i in range(2)]
        q_in = dram.tile([B, D], f32)
        q_out = dram.tile([B, D], f32)

        # ---- layer-2 gather pre-staging --------------------------------
        # Descriptors for the first NPRE sub-calls are generated on the Q7
        # while layer 1 computes; their DMAs fire (trigger_dma) once h1full
        # exists. Pre-staged destinations are fresh pool slots, so no WAR
        # edge holds the preps back.
        gh_pool = ctx.enter_context(tc.tile_pool(name="gh", bufs=GH_BUFS))

        ident = const.tile([64, 64], f32)
        make_identity(nc, ident)

        def bn_vec(star, g_sb, be_sb, st):
            """star=[1,1024] (sum|sumsq) -> st=[1,1024] (scale|shift)."""
            a = vp.tile([1, D], f32, tag="bnv_a")   # mean (live to the end)
            b = vp.tile([1, D], f32, tag="bnv_b")
            c = vp.tile([1, D], f32, tag="bnv_c")
            nc.vector.tensor_scalar_mul(a, star[:, 0:D], 1.0 / N)
            nc.vector.tensor_scalar_mul(b, star[:, D:2 * D], 1.0 / N)
            nc.vector.tensor_mul(c, a, a)
            nc.vector.tensor_sub(b, b, c)                    # var
            nc.scalar.activation(c, b, AF.Sqrt, bias=eps1)   # sd
            nc.vector.reciprocal(b, c)                       # rstd
            nc.vector.tensor_mul(st[:, 0:D], b, g_sb)        # s
            nc.vector.tensor_mul(c, a, st[:, 0:D])           # mean*s
            nc.vector.tensor_sub(st[:, D:2 * D], be_sb, c)   # t

        def bn_broadcast(st, stb):
            """st=[1,1024] -> stb=[128,1024] via ones matmul."""
            sb_ps = psA.tile([128, D], f32, tag="work")
            tb_ps = psA.tile([128, D], f32, tag="work")
            nc.tensor.matmul(sb_ps, ones1, st[:, 0:D], start=True, stop=True)
            nc.tensor.matmul(tb_ps, ones1, st[:, D:2 * D], start=True, stop=True)
            nc.vector.tensor_copy(stb[:, 0:D], sb_ps)
            nc.vector.tensor_copy(stb[:, D:2 * D], tb_ps)

        # =================== Layer 1 ===================
        with ExitStack() as l1ctx:
            l1p = l1ctx.enter_context(tc.tile_pool(name="l1p", bufs=1))
            l1t = l1ctx.enter_context(tc.tile_pool(name="l1t", bufs=2))
            gx_pool = l1ctx.enter_context(tc.tile_pool(name="gx", bufs=3))

            zT = l1p.tile([KAUG, NPAD], f32)
            nc.sync.dma_start(out=zT, in_=xTaug)
            invb_sb = l1p.tile([DIN, NPAD], f32)
            nc.sync.dma_start(
                out=invb_sb,
                in_=bass.AP(tensor=invdegb.tensor, offset=invdegb.offset,
                            ap=[[0, DIN]] + list(invdegb.ap[1:])))

            for bidx in range(NBLK):
                Gxb = gx_pool.tile([128, T_BLK, XP], bf, tag="gx")
                nc.sync.dma_start(
                    out=Gxb, in_=Gx_d[:, bidx * T_BLK:(bidx + 1) * T_BLK, :])
                zt_ps = psA.tile([XP, 128], f32, tag="work")
                for t in range(T_BLK):
                    nc.tensor.matmul(zt_ps, Gxb[:, t, :],
                                     S_sb[:, bidx * T_BLK + t, :],
                                     start=(t == 0), stop=(t == T_BLK - 1))
                tmp = l1t.tile([DIN, 128], f32, tag="ztmp")
                nc.vector.tensor_mul(tmp, zt_ps[0:DIN, :],
                                     invb_sb[:, ts(bidx, 128)])
                nc.vector.tensor_add(zT[0:DIN, ts(bidx, 128)],
                                     zT[0:DIN, ts(bidx, 128)], tmp)

            if stage == 12:
                dbg_out(zT[0:21, 0:10], f32, p=21)
            if stage <= 12:
                raise _StageDone()

            u_sb = l1p.tile([128, NBLK, D], f32)
            sum_ps = psStat.tile([1, D], f32, tag="sum")
            ssq_ps = psStat.tile([1, D], f32, tag="ssq")
            for bidx in range(NBLK):
                u_ps = psA.tile([128, D], f32, tag="work")
                nc.tensor.matmul(u_ps, zT[:, ts(bidx, 128)], Waug_sb,
                                 start=True, stop=True)
                nc.vector.tensor_copy(u_sb[:, bidx, :], u_ps)
                usq = sq_pool.tile([128, D], f32, tag="usq")
                nc.scalar.square(usq, u_ps)
                nc.tensor.matmul(sum_ps, mask_sb[:, bidx:bidx + 1],
                                 u_sb[:, bidx, :],
                                 start=(bidx == 0), stop=(bidx == NBLK - 1))
                nc.tensor.matmul(ssq_ps, mask_sb[:, bidx:bidx + 1], usq,
                                 start=(bidx == 0), stop=(bidx == NBLK - 1))

            if stage == 14:
                dbg_out(u_sb[0:64, 0, 0:10], f32)
            if stage <= 14:
                raise _StageDone()

            stats_sb = l1p.tile([1, 2 * D], f32)
            nc.vector.tensor_copy(stats_sb[:, 0:D], sum_ps)
            nc.vector.tensor_copy(stats_sb[:, D:2 * D], ssq_ps)
            nc.sync.dma_start(out=bnc_in[0][:], in_=stats_sb)
            nc.gpsimd.collective_compute(
                "AllReduce", ALU.add, replica_groups=RG,
                ins=[bnc_in[0].opt()], outs=[bnc_out[0].opt()])
            star1 = l1p.tile([1, 2 * D], f32)
            nc.sync.dma_start(out=star1, in_=bnc_out[0][:])

            if stage == 15:
                dbg_out(star1[0:1, 0:10], f32, p=1)
            if stage <= 15:
                raise _StageDone()

            st1 = l1p.tile([1, 2 * D], f32)
            bn_vec(star1, g1_sb, be1_sb, st1)
            stb1 = l1p.tile([128, 2 * D], f32)
            bn_broadcast(st1, stb1)

            if stage == 16:
                dbg_out(stb1[0:64, 0:10], f32)
            if stage <= 16:
                raise _StageDone()

            for bidx in range(NBLK):
                t1 = l1t.tile([128, D], f32, tag="ap1")
                nc.vector.tensor_mul(t1, u_sb[:, bidx, :], stb1[:, 0:D])
                t2 = l1t.tile([128, D], f32, tag="ap2")
                nc.vector.tensor_add(t2, t1, stb1[:, D:2 * D])
                nc.scalar.activation(h1_sb[:, bidx, :], t2, AF.Relu)
                nb = min(128, NLOC - bidx * 128)
                nc.sync.dma_start(
                    out=h1loc[bidx * 128:bidx * 128 + nb, :],
                    in_=h1_sb[0:nb, bidx, :])

            if stage == 1:
                dbg_out(h1_sb[0:64, 0, 0:10], bf)
            if stage <= 1:
                raise _StageDone()
            nc.gpsimd.collective_compute(
                "AllGather", ALU.bypass, replica_groups=RG,
                ins=[h1loc.opt()], outs=[h1full.opt()])

        if stage == 2:
            h1chk = const.tile([64, 10], bf, name="h1chk")
            nc.sync.dma_start(out=h1chk, in_=h1full[0:64, 0:10])
            dbg_out(h1chk, bf)
        if stage <= 2:
            raise _StageDone()

        # =================== Layer 2 ===================
        l2ctx = ctx.enter_context(ExitStack())
        l2p = l2ctx.enter_context(tc.tile_pool(name="l2p", bufs=1))
        l2t = l2ctx.enter_context(tc.tile_pool(name="l2t", bufs=2))

        gh_dma_sems = [nc.alloc_semaphore(f"gh_dma{k}")
                       for k in range(NCALL)]
        u2_sb = l2p.tile([128, NBLK, D], f32)
        sum2_ps = psStat.tile([1, D], f32, tag="sum")
        ssq2_ps = psStat.tile([1, D], f32, tag="ssq")
        for bidx in range(NBLK):
            gts = []
            for ci in range(CPB):
                k = bidx * CPB + ci
                gt = gh_pool.tile([128, SUB, D], bf, tag="gh", name=f"gh{k}")
                gts.append(gt)
                nc.gpsimd.dma_gather(
                    gt, h1full[:],
                    idx_sb[:, k * SUB * 8:(k + 1) * SUB * 8],
                    SUB * 128, SUB * 128, D,
                    prepare_only=True, sem=gh_dma_sems[k])
                nc.gpsimd.trigger_dma(count=None)
            agg_ps = psA.tile([128, D], f32, tag="work")
            for t in range(T_BLK):
                if t % SUB == 0:
                    nc.tensor.wait_ge(gh_dma_sems[bidx * CPB + t // SUB], 16)
                nc.tensor.matmul(agg_ps, S_sb[:, bidx * T_BLK + t, :],
                                 gts[t // SUB][:, t % SUB, :],
                                 start=(t == 0), stop=(t == T_BLK - 1))
            nc.vector.scalar_tensor_tensor(
                u2_sb[:, bidx, :], agg_ps, invpp_sb[:, bidx:bidx + 1],
                h1_sb[:, bidx, :], op0=ALU.mult, op1=ALU.add)
            usq2 = sq_pool.tile([128, D], f32, tag="usq")
            nc.scalar.square(usq2, u2_sb[:, bidx, :])
            nc.tensor.matmul(sum2_ps, mask_sb[:, bidx:bidx + 1],
                             u2_sb[:, bidx, :],
                             start=(bidx == 0), stop=(bidx == NBLK - 1))
            nc.tensor.matmul(ssq2_ps, mask_sb[:, bidx:bidx + 1], usq2,
                             start=(bidx == 0), stop=(bidx == NBLK - 1))

        sctx.close()

        if stage == 3:
            dbg_out(u2_sb[0:64, 0, 0:10], f32)
        if stage <= 3:
            raise _StageDone()
        stats2_sb = l2p.tile([1, 2 * D], f32)
        nc.vector.tensor_copy(stats2_sb[:, 0:D], sum2_ps)
        nc.vector.tensor_copy(stats2_sb[:, D:2 * D], ssq2_ps)
        nc.sync.dma_start(out=bnc_in[1][:], in_=stats2_sb)
        nc.gpsimd.collective_compute(
            "AllReduce", ALU.add, replica_groups=RG,
            ins=[bnc_in[1].opt()], outs=[bnc_out[1].opt()])
        star2 = l2p.tile([1, 2 * D], f32)
        nc.sync.dma_start(out=star2, in_=bnc_out[1][:])

        # ---- head constants + early head matmuls (overlap AllReduce) ----
        hp = ctx.enter_context(tc.tile_pool(name="hp", bufs=1, side="right"))
        hv = ctx.enter_context(tc.tile_pool(name="hv", bufs=2, side="right"))
        W1_sb = hp.tile([128, 12, DH], bf)
        nc.sync.dma_start(out=W1_sb, in_=Wfc1t)
        W2_sb = hp.tile([128, 2, DH], bf)
        nc.sync.dma_start(out=W2_sb, in_=Wfc2t)
        W3_sb = hp.tile([128, 2, DH], bf)
        nc.sync.dma_start(out=W3_sb, in_=Wfc3t)
        W4_sb = hp.tile([128, 2, 1], bf)
        nc.sync.dma_start(out=W4_sb, in_=Wfc4t)
        gbT_sb = hp.tile([128, 2], f32)
        nc.sync.dma_start(out=gbT_sb, in_=gbT)
        bbT_sb = hp.tile([128, 2], f32)
        nc.sync.dma_start(out=bbT_sb, in_=bbT)
        gb2T_sb = hp.tile([128, 2], f32)
        nc.sync.dma_start(out=gb2T_sb, in_=gb2T)
        bb2T_sb = hp.tile([128, 2], f32)
        nc.sync.dma_start(out=bb2T_sb, in_=bb2T)
        gb3T_sb = hp.tile([128, 2], f32)
        nc.sync.dma_start(out=gb3T_sb, in_=gb3T)
        bb3T_sb = hp.tile([128, 2], f32)
        nc.sync.dma_start(out=bb3T_sb, in_=bb3T)
        b4_sb = hp.tile([128, 1], f32)
        nc.sync.dma_start(out=b4_sb, in_=b4rep)
        pgT_sb = hp.tile([128, 4, B], bf)
        nc.sync.dma_start(out=pgT_sb, in_=pgT)
        nghT_sb = hp.tile([128, 4, B * KCAND], bf)
        nc.sync.dma_start(out=nghT_sb, in_=neighT)

        def rep10(sl, nchunk):
            # [128, 64] slice -> [128, 320] with each column repeated 10x
            gstep = sl.ap[1][0]
            return bass.AP(tensor=sl.tensor, offset=sl.offset + nchunk * 32 * gstep,
                           ap=[list(sl.ap[0]), [gstep, 32], [0, 10]])

        HT_ps = [[psH.tile([128, 320], f32, name=f"ht{m}{n}", tag=f"hm{m}{n}")
                  for n in range(2)] for m in range(2)]
        for m in range(2):
            for n in range(2):
                for kt in range(4, 12):
                    if kt < 8:
                        rhs = rep10(pgT_sb[:, kt - 4, :], n)
                    else:
                        rhs = nghT_sb[:, kt - 8, n * 320:(n + 1) * 320]
                    nc.tensor.matmul(HT_ps[m][n],
                                     W1_sb[:, kt, ts(m, 128)], rhs,
                                     start=(kt == 4), stop=False)

        st2 = l2p.tile([1, 2 * D], f32)
        bn_vec(star2, g2_sb, be2_sb, st2)
        stb2 = l2p.tile([128, 2 * D], f32)
        bn_broadcast(st2, stb2)

        for bidx in range(NBLK):
            t1 = l2t.tile([128, D], f32, tag="ap1")
            nc.vector.tensor_mul(t1, u2_sb[:, bidx, :], stb2[:, 0:D])
            t2 = l2t.tile([128, D], f32, tag="ap2")
            nc.vector.tensor_add(t2, t1, stb2[:, D:2 * D])
            nc.scalar.activation(h2_sb[:, bidx, :], t2, AF.Relu)

        if stage == 4:
            dbg_out(h2_sb[0:64, 0, 0:10], f32)
        if stage <= 4:
            raise _StageDone()

        # =================== Readout ===================
        qs_ps = psStat.tile([B, D], f32, tag="sum")
        for bidx in range(NBLK):
            nc.tensor.matmul(qs_ps, C_sb[:, bidx, :], h2_sb[:, bidx, :],
                             start=(bidx == 0), stop=(bidx == NBLK - 1))
        qs_sb = l2p.tile([B, D], f32)
        nc.vector.tensor_copy(qs_sb, qs_ps)
        nc.sync.dma_start(out=q_in[:], in_=qs_sb)
        nc.gpsimd.collective_compute(
            "AllReduce", ALU.add, replica_groups=RG,
            ins=[q_in.opt()], outs=[q_out.opt()])
        qar_sb = l2p.tile([B, D], f32)
        nc.sync.dma_start(out=qar_sb, in_=q_out[:])
        qemb_sb = l2p.tile([B, D], f32)
        nc.scalar.activation(qemb_sb, qar_sb, AF.Copy, scale=invcnt_sb)

        if stage == 5:
            dbg_out(qemb_sb[0:64, 0:10], f32)
        if stage <= 5:
            raise _StageDone()
        qT_sb = keep.tile([128, 4, B], bf)
        for j in range(4):
            qT_ps = psA.tile([128, B], f32, tag="work")
            nc.tensor.transpose(qT_ps, qemb_sb[:, ts(j, 128)], ident)
            nc.vector.tensor_copy(qT_sb[:, j, :], qT_ps)

        if stage == 6:
            dbg_out(qT_sb[0:64, 0, 0:10], bf)
        if stage <= 6:
            raise _StageDone()

        l2ctx.close()

        # =================== Head (bf16, feature-major) ===================
        # finish MM1 with the qemb k-tiles
        for m in range(2):
            for n in range(2):
                for kt in range(4):
                    rhs = rep10(qT_sb[:, kt, :], n)
                    nc.tensor.matmul(HT_ps[m][n],
                                     W1_sb[:, kt, ts(m, 128)], rhs,
                                     start=False, stop=(kt == 3))
        HT = hp.tile([128, 2, 640], f32)
        for m in range(2):
            for n in range(2):
                nc.vector.tensor_copy(HT[:, m, n * 320:(n + 1) * 320],
                                      HT_ps[m][n])

        def head_bn_relu(pre_sb, gT, bT_, out_sb):
            """pre_sb [128, 2, 640] f32; BN over 640 rows + ReLU -> bf16 out_sb."""
            for m in range(2):
                cat = pre_sb[:, m, :]
                sums = hv.tile([128, 1], f32, tag="hsum")
                nc.vector.tensor_reduce(sums, cat, mybir.AxisListType.X,
                                        ALU.add)
                sqj = hv.tile([128, 640], f32, tag="hsq")
                ssq = hv.tile([128, 1], f32, tag="hssq")
                nc.scalar.activation(sqj, cat, AF.Square, accum_out=ssq)
                mean = hv.tile([128, 1], f32, tag="hmean")
                nc.vector.tensor_scalar_mul(mean, sums, 1.0 / 640.0)
                ex2 = hv.tile([128, 1], f32, tag="hex2")
                nc.vector.tensor_scalar_mul(ex2, ssq, 1.0 / 640.0)
                msq = hv.tile([128, 1], f32, tag="hmsq")
                nc.vector.tensor_mul(msq, mean, mean)
                var = hv.tile([128, 1], f32, tag="hvar")
                nc.vector.tensor_sub(var, ex2, msq)
                sd = hv.tile([128, 1], f32, tag="hsd")
                nc.scalar.activation(sd, var, AF.Sqrt, bias=eps128)
                rstd = hv.tile([128, 1], f32, tag="hrstd")
                nc.vector.reciprocal(rstd, sd)
                s = hv.tile([128, 1], f32, tag="hs")
                nc.vector.tensor_mul(s, rstd, gT[:, m:m + 1])
                ms = hv.tile([128, 1], f32, tag="hms")
                nc.vector.tensor_mul(ms, mean, s)
                t = hv.tile([128, 1], f32, tag="ht")
                nc.vector.tensor_sub(t, bT_[:, m:m + 1], ms)
                nc.scalar.activation(out_sb[:, m, :], cat, AF.Relu,
                                     scale=s, bias=t)

        def head_layer_mm(rhs_in, W_sb, pre_sb):
            for m in range(2):
                for n in range(2):
                    ps = psH.tile([128, 320], f32, tag=f"hm{m}{n}")
                    for kt in range(2):
                        nc.tensor.matmul(ps, W_sb[:, kt, ts(m, 128)],
                                         rhs_in[:, kt, n * 320:(n + 1) * 320],
                                         start=(kt == 0), stop=(kt == 1))
                    nc.vector.tensor_copy(pre_sb[:, m, n * 320:(n + 1) * 320],
                                          ps)

        H1h = hp.tile([128, 2, 640], bf)
        head_bn_relu(HT, gbT_sb, bbT_sb, H1h)

        H2p = hp.tile([128, 2, 640], f32)
        head_layer_mm(H1h, W2_sb, H2p)
        H2h = hp.tile([128, 2, 640], bf)
        head_bn_relu(H2p, gb2T_sb, bb2T_sb, H2h)

        H3p = hp.tile([128, 2, 640], f32)
        head_layer_mm(H2h, W3_sb, H3p)
        H3h = hp.tile([128, 2, 640], bf)
        head_bn_relu(H3p, gb3T_sb, bb3T_sb, H3h)

        pred_sb = hp.tile([128, 5], f32)
        for rr in range(5):
            pr_ps = psA.tile([128, 1], f32, tag="work")
            for kt in range(2):
                nc.tensor.matmul(pr_ps, H3h[:, kt, ts(rr, 128)],
                                 W4_sb[:, kt, :],
                                 start=(kt == 0), stop=(kt == 1))
            nc.scalar.activation(pred_sb[:, rr:rr + 1], pr_ps, AF.Sigmoid,
                                 bias=b4_sb)

        nc.sync.dma_start(
            out=bass.AP(tensor=outp.tensor, offset=outp.offset,
                        ap=[[1, 128], [128, 5]]),
            in_=pred_sb)
     except _StageDone:
        pass
    nc.compile()
    return nc


# ---------------------------------------------------------------------------
# Entry point
# ---------------------------------------------------------------------------

def kernel(**inputs) -> np.ndarray:
    global LAST_EXEC_NS
    from concourse.bass_utils import run_bass_kernel_spmd

    in_maps, T_BLK = preprocess(**inputs)
    nc = build_nc(T_BLK)

    trace = bool(int(os.environ.get("GNN_TRACE", "0")))
    kw = {}
    if trace:
        kw = dict(trace=True, trace_cores=list(range(NCORES)),
                  stitch_traces=False)
    try:
        res = run_bass_kernel_spmd(nc, in_maps, core_ids=list(range(NCORES)),
                                   **kw)
    except Exception:
        if not trace:
            raise
        res = run_bass_kernel_spmd(nc, in_maps, core_ids=list(range(NCORES)))
    LAST_EXEC_NS = res.exec_time_ns
    return np.asarray(res.results[0]["outp"], np.float32)



# revision 18
# speedup vs baseline: 1.4854x; 1.4854x over previous
"""Trainium2 Bass kernel for nn_Model_40827959116312 (GIN message passing + MLP head).

Self-contained: builds per-core graph structures on host (numpy), compiles a
Bass/Tile SPMD program for 8 NeuronCores, runs via run_bass_kernel_spmd, and
returns the full [64, 10] output.

Sharding: data-parallel over destination nodes (1250 per core, 10 blocks of
128). Layer-1 aggregation runs on host-pregathered x rows (the projection
commutes with the mean-aggregation, so only the 21-wide augmented features are
aggregated). Layer-2 gathers fp8 h1 rows via dma_gather with per-block
source deduplication. All gather descriptor generation (the serial ~8.5us/call
Q7 cost) is pre-staged with prepare_only during layer 1 + the collectives,
reading h1full through an address-alias tensor so Tile doesn't pin the
AllGather dependency on the preps; a trigger_dma ordered after the AllGather
(WAW via signals_writable) fires the DMAs, and consumers wait per-call DMA
semaphores. Segment-sums are one-hot/count matmuls accumulating in PSUM
(count matrices in fp8 — exact for small integer counts). BatchNorm
statistics are [2,512] AllReduces; h1 is AllGathered in fp8. The per-graph
readout uses a compact per-core slot layout + small AllGather + combine
matmul instead of a [64,512] AllReduce. The MLP head is replicated on every
core (feature-major bf16, fused BN+ReLU activations)."""

import os
import numpy as np
import ml_dtypes

bf16 = ml_dtypes.bfloat16
f8e4 = ml_dtypes.float8_e4m3

# Problem constants (from spec).
N, E, B, KCAND = 10000, 160000, 64, 10
DIN, D, DH = 20, 512, 256
NCORES = 8
NLOC = N // NCORES            # 1250
NBLK = (NLOC + 127) // 128    # 10
NPAD = NBLK * 128             # 1280
XP = 32                       # padded x feature width (host-pregathered)
KAUG = DIN + 1                # 21 (features + bias row)
BN_EPS = 1e-5
SUB = 8                       # gather sub-call size: SUB*128 = 1024 indices
PRE_AR1 = 6                   # preps emitted before the BN1-stats AllReduce
PRE_AG = 10                   # preps emitted before the h1 AllGather dispatch
PRE_T1 = 99                   # preps before the first trigger (>=NCALL: one)

LAST_EXEC_NS = None           # set by kernel() when profiling succeeds


# ---------------------------------------------------------------------------
# Host-side preprocessing
# ---------------------------------------------------------------------------

def preprocess(x, pg_emb, neigh_emb, W_init, b_init, g1, be1, g2, be2,
               W_fc, b_fc, W_fc2, b_fc2, W_fc3, b_fc3, W_fc4, b_fc4,
               gb, bb, gb2, bb2, gb3, bb3, edge_src, edge_dst, node2graph):
    """Build per-core input maps + the uniform per-block tile count T_BLK."""
    x = np.asarray(x, np.float32)
    edge_src = np.asarray(edge_src, np.int64)
    edge_dst = np.asarray(edge_dst, np.int64)
    node2graph = np.asarray(node2graph, np.int64)

    deg = np.bincount(edge_dst, minlength=N).astype(np.float64)
    invdeg = (1.0 / np.maximum(deg, 1.0)).astype(np.float32)
    r = (deg > 0).astype(np.float32)

    # Per (core, block): unique sources + count matrices.
    per_core = []
    t_blk = SUB
    for c in range(NCORES):
        lo = c * NLOC
        sel = (edge_dst >= lo) & (edge_dst < lo + NLOC)
        s_c = edge_src[sel]
        d_c = edge_dst[sel] - lo
        blocks = []
        for bidx in range(NBLK):
            bsel = (d_c >> 7) == bidx
            s_b = s_c[bsel]
            d_b = d_c[bsel] - (bidx << 7)
            uniq, inv = np.unique(s_b, return_inverse=True)
            blocks.append((uniq, inv, d_b))
            t_blk = max(t_blk, (len(uniq) + 127) // 128)
        per_core.append(blocks)

    T_BLK = ((t_blk + SUB - 1) // SUB) * SUB   # multiple of SUB
    NT = NBLK * T_BLK              # total edge tiles per core
    NU = T_BLK * 128               # padded unique srcs per block

    xpad = np.zeros((N, XP), np.float32)
    xpad[:, :DIN] = x
    xpad_bf = xpad.astype(bf16)

    W_aug = np.zeros((KAUG, D), np.float32)
    W_aug[:DIN] = np.asarray(W_init, np.float32)
    W_aug[DIN] = np.asarray(b_init, np.float32)

    cnt = np.bincount(node2graph, minlength=B).astype(np.float64)
    inv_cnt = (1.0 / np.maximum(cnt, 1.0)).astype(np.float32).reshape(B, 1)

    # Compact per-core readout slots: core c's nodes span graphs
    # [g_lo[c], g_hi[c]]; SLOTS = max span so the combine matrix is uniform.
    g_lo = [int(node2graph[c * NLOC]) for c in range(NCORES)]
    g_hi = [int(node2graph[(c + 1) * NLOC - 1]) for c in range(NCORES)]
    SLOTS = max(h - l + 1 for l, h in zip(g_lo, g_hi))
    Msel = np.zeros((NCORES * SLOTS, B), np.float32)
    for c in range(NCORES):
        for j in range(SLOTS):
            g = g_lo[c] + j
            if g <= g_hi[c] and g < B:
                Msel[c * SLOTS + j, g] = 1.0

    # Head weights, feature-major bf16 layouts.
    W_fc = np.asarray(W_fc, np.float32)      # [1536, 256]
    Wfc1t = np.ascontiguousarray(
        W_fc.reshape(12, 128, DH).transpose(1, 0, 2)).astype(bf16)
    Wfc2t = np.ascontiguousarray(
        np.asarray(W_fc2, np.float32).reshape(2, 128, DH).transpose(1, 0, 2)).astype(bf16)
    Wfc3t = np.ascontiguousarray(
        np.asarray(W_fc3, np.float32).reshape(2, 128, DH).transpose(1, 0, 2)).astype(bf16)
    Wfc4t = np.ascontiguousarray(
        np.asarray(W_fc4, np.float32).reshape(2, 128, 1).transpose(1, 0, 2)).astype(bf16)

    def ppart(v):  # [256] -> [128, 2] (dh = kt*128 + p)
        return np.ascontiguousarray(np.asarray(v, np.float32).reshape(2, 128).T)

    pgT = np.ascontiguousarray(
        np.asarray(pg_emb, np.float32).T.reshape(4, 128, B).transpose(1, 0, 2)).astype(bf16)
    neighT = np.ascontiguousarray(
        np.asarray(neigh_emb, np.float32).reshape(B * KCAND, D).T
        .reshape(4, 128, B * KCAND).transpose(1, 0, 2)).astype(bf16)
    b4rep = np.full((128, 1), float(np.asarray(b_fc4).reshape(-1)[0]), np.float32)

    shared = dict(
        W_aug=W_aug,
        bn1g=np.asarray(g1, np.float32).reshape(1, D),
        bn1b=np.asarray(be1, np.float32).reshape(1, D),
        bn2g=np.asarray(g2, np.float32).reshape(1, D),
        bn2b=np.asarray(be2, np.float32).reshape(1, D),
        invcnt=inv_cnt,
        Msel=Msel,
        Wfc1t=Wfc1t, Wfc2t=Wfc2t, Wfc3t=Wfc3t, Wfc4t=Wfc4t,
        gbT=ppart(gb), bbT=ppart(bb),
        gb2T=ppart(gb2), bb2T=ppart(bb2),
        gb3T=ppart(gb3), bb3T=ppart(bb3),
        b4rep=b4rep,
        pgT=pgT, neighT=neighT,
    )

    in_maps = []
    for c in range(NCORES):
        lo = c * NLOC
        S = np.zeros((NT, 128, 128), np.float32)
        idx_flat = np.zeros(NT * 128, np.int64)
        for bidx in range(NBLK):
            uniq, inv, d_b = per_core[c][bidx]
            Sb = np.zeros((NU, 128), np.float32)
            np.add.at(Sb, (inv, d_b), 1.0)
            S[bidx * T_BLK:(bidx + 1) * T_BLK] = Sb.reshape(T_BLK, 128, 128)
            idx_flat[bidx * NU: bidx * NU + len(uniq)] = uniq
        # wrap: slot i lives at [i % 16, i // 16], tiled over 128 partitions
        idx_w = np.tile(idx_flat.reshape(-1, 16).T, (8, 1)).astype(np.int16)
        # host-pregathered x rows in the device SBUF layout [128, NT, XP]
        Gx = np.ascontiguousarray(xpad_bf[idx_flat.reshape(NT, 128).T])

        nloc_ids = np.arange(NPAD)
        real = nloc_ids < NLOC
        gids = np.minimum(lo + nloc_ids, N - 1)

        xTaug = np.zeros((KAUG, NPAD), np.float32)
        xTaug[:DIN, :NLOC] = x[lo:lo + NLOC].T
        xTaug[DIN, :NLOC] = 1.0 + r[lo:lo + NLOC]

        invdegb = np.zeros((1, NPAD), np.float32)
        invdegb[0, :NLOC] = invdeg[lo:lo + NLOC]

        invdeg_pp = np.where(real, invdeg[gids], 0.0).reshape(NBLK, 128).T.copy()
        maskn = real.astype(np.float32).reshape(NBLK, 128).T.copy()

        # per-graph readout into SLOTS compact rows (graph g -> g - g_lo[c])
        Cro = np.zeros((NBLK, 128, SLOTS), np.float32)
        n2g_loc = node2graph[lo:lo + NLOC]
        Cro.reshape(NPAD, SLOTS)[nloc_ids[real], n2g_loc - g_lo[c]] = 1.0

        m = dict(shared)
        m.update(
            idx=idx_w,
            S=S.astype(f8e4),
            Gx=Gx,
            xTaug=xTaug,
            invdegb=invdegb,
            invdeg_pp=invdeg_pp,
            maskn=maskn,
            Cro=Cro,
        )
        in_maps.append(m)

    return in_maps, T_BLK, SLOTS


# ---------------------------------------------------------------------------
# Device program
# ---------------------------------------------------------------------------

def build_nc(T_BLK, SLOTS):
    from contextlib import ExitStack

    import concourse.bass as bass
    import concourse.mybir as mybir
    import concourse.tile as tile
    from concourse import bacc
    from concourse.bass import ts
    from concourse.masks import make_identity

    f32 = mybir.dt.float32
    bf = mybir.dt.bfloat16
    f8 = mybir.dt.float8e4
    i16 = mybir.dt.int16
    AF = mybir.ActivationFunctionType
    ALU = mybir.AluOpType

    NT = NBLK * T_BLK
    NCALL = NT // SUB              # layer-2 gather sub-calls
    CPB = T_BLK // SUB             # sub-calls per block
    RG = [list(range(NCORES))]

    nc = bacc.Bacc("TRN2", target_bir_lowering=False, debug=False,
                   num_devices=NCORES, num_swdge_queues=2)

    def din(name, shape, dt):
        return nc.dram_tensor(name, list(shape), dt, kind="ExternalInput").ap()

    idx = din("idx", (128, NT * 8), i16)
    S = din("S", (NT, 128, 128), f8)
    Gx_d = din("Gx", (128, NT, XP), bf)
    xTaug = din("xTaug", (KAUG, NPAD), f32)
    invdegb = din("invdegb", (1, NPAD), f32)
    invdeg_pp = din("invdeg_pp", (128, NBLK), f32)
    maskn = din("maskn", (128, NBLK), f32)
    W_aug = din("W_aug", (KAUG, D), f32)
    bn1g = din("bn1g", (1, D), f32)
    bn1b = din("bn1b", (1, D), f32)
    bn2g = din("bn2g", (1, D), f32)
    bn2b = din("bn2b", (1, D), f32)
    Cro = din("Cro", (NBLK, 128, SLOTS), f32)
    Msel = din("Msel", (NCORES * SLOTS, B), f32)
    invcnt = din("invcnt", (B, 1), f32)
    Wfc1t = din("Wfc1t", (128, 12, DH), bf)
    Wfc2t = din("Wfc2t", (128, 2, DH), bf)
    Wfc3t = din("Wfc3t", (128, 2, DH), bf)
    Wfc4t = din("Wfc4t", (128, 2, 1), bf)
    gbT = din("gbT", (128, 2), f32)
    bbT = din("bbT", (128, 2), f32)
    gb2T = din("gb2T", (128, 2), f32)
    bb2T = din("bb2T", (128, 2), f32)
    gb3T = din("gb3T", (128, 2), f32)
    bb3T = din("bb3T", (128, 2), f32)
    b4rep = din("b4rep", (128, 1), f32)
    pgT = din("pgT", (128, 4, B), bf)
    neighT = din("neighT", (128, 4, B * KCAND), bf)

    outp = nc.dram_tensor("outp", [B, KCAND], f32, kind="ExternalOutput").ap()

    # h1full: AllGather output. The gather preps read it through an address
    # alias ("h1full_r") so Tile doesn't order the descriptor generation
    # after the AllGather; the trigger is ordered after the AllGather via a
    # WAW edge (signals_writable) instead.
    h1full_h = nc.dram_tensor("h1full", [N, D], f8, kind="Internal",
                              addr_space="Shared")
    h1full = h1full_h.ap()
    _mloc = nc.lookup_mloc(h1full_h)
    _amls = nc._tensor("h1full_r", [N, D], f8, kind="Internal", type="DRAM",
                       addr_space="Shared")
    _amls.memorylocations[0].addr = _mloc.addr
    _amls.memorylocations[0].allocated = _mloc.allocated
    h1full_r = bass.DRamTensorHandle("h1full_r", [N, D], f8).ap()

    with tile.TileContext(nc) as tc, ExitStack() as ctx:
        const = ctx.enter_context(tc.tile_pool(name="const", bufs=1))
        dram = ctx.enter_context(tc.tile_pool(name="dram", bufs=1, space="DRAM"))
        # PSUM static budget (8 banks): work(2) + sum(1) + ssq(1) + hm(4).
        psA = ctx.enter_context(tc.tile_pool(name="psA", bufs=2, space="PSUM"))
        psStat = ctx.enter_context(tc.tile_pool(name="psStat", bufs=1, space="PSUM"))
        psH = ctx.enter_context(tc.tile_pool(name="psH", bufs=1, space="PSUM"))
        vp = ctx.enter_context(tc.tile_pool(name="vp", bufs=1))
        sq_pool = ctx.enter_context(tc.tile_pool(name="sq", bufs=2))
        keep = ctx.enter_context(tc.tile_pool(name="keep", bufs=1))

        # ---- collective warmup (absorbs first-collective setup + skew) ----
        wu_in = dram.tile([1, 16], f32)
        wu_out = dram.tile([1, 16], f32)
        wu_sb = const.tile([1, 16], f32)
        nc.vector.memset(wu_sb, 1.0)
        nc.sync.dma_start(out=wu_in[:], in_=wu_sb)
        nc.gpsimd.collective_compute(
            "AllReduce", ALU.add, replica_groups=RG,
            ins=[wu_in.opt()], outs=[wu_out.opt()])

        # ---- resident loads (idx first: the gather preps depend on it) ----
        sctx = ctx.enter_context(ExitStack())
        spool = sctx.enter_context(tc.tile_pool(name="spool", bufs=1, side="right"))
        idx_sb = spool.tile([128, NT * 8], i16)
        nc.sync.dma_start(out=idx_sb, in_=idx)
        S_sb = spool.tile([128, NT, 128], f8)
        for bidx in range(NBLK):
            nc.sync.dma_start(
                out=S_sb[:, bidx * T_BLK:(bidx + 1) * T_BLK, :],
                in_=S[bidx * T_BLK:(bidx + 1) * T_BLK, :, :]
                .rearrange("t e d -> e t d"))
        Waug_sb = const.tile([KAUG, D], f32)
        nc.sync.dma_start(out=Waug_sb, in_=W_aug)
        invpp_sb = const.tile([128, NBLK], f32)
        nc.sync.dma_start(out=invpp_sb, in_=invdeg_pp)
        mask_sb = const.tile([128, NBLK], f32)
        nc.sync.dma_start(out=mask_sb, in_=maskn)
        mask_bf = const.tile([128, NBLK], bf)
        nc.vector.tensor_copy(mask_bf, mask_sb)
        g1_sb = const.tile([1, D], f32)
        nc.sync.dma_start(out=g1_sb, in_=bn1g)
        be1_sb = const.tile([1, D], f32)
        nc.sync.dma_start(out=be1_sb, in_=bn1b)
        g2_sb = const.tile([1, D], f32)
        nc.sync.dma_start(out=g2_sb, in_=bn2g)
        be2_sb = const.tile([1, D], f32)
        nc.sync.dma_start(out=be2_sb, in_=bn2b)
        C_f32 = const.tile([128, NBLK, SLOTS], f32)
        nc.sync.dma_start(out=C_f32, in_=Cro.rearrange("b p g -> p b g"))
        C_sb = const.tile([128, NBLK, SLOTS], bf)
        nc.vector.tensor_copy(C_sb, C_f32)
        Msel_sb = const.tile([NCORES * SLOTS, B], f32)
        nc.sync.dma_start(out=Msel_sb, in_=Msel)
        invcnt_sb = const.tile([B, 1], f32)
        nc.sync.dma_start(out=invcnt_sb, in_=invcnt)

        ones1 = const.tile([1, 128], f32)
        nc.vector.memset(ones1, 1.0)
        eps1 = const.tile([1, 1], f32)
        nc.vector.memset(eps1, BN_EPS)
        eps128 = const.tile([128, 1], f32)
        nc.vector.memset(eps128, BN_EPS)

        h1_sb = keep.tile([128, NBLK, D], bf)      # bf16 h1 (local rows)
        h1f8_sb = keep.tile([128, NBLK, D], f8)    # fp8 h1 (for the AllGather)
        h2_sb = keep.tile([128, NBLK, D], bf)      # bf16 h2 (local rows)

        h1loc = dram.tile([NLOC, D], f8)
        bnc_in = [dram.tile([1, 2 * D], f32, name=f"bi{i}") for i in range(2)]
        bnc_out = [dram.tile([1, 2 * D], f32, name=f"bo{i}") for i in range(2)]
        q_in = dram.tile([SLOTS, D], f32)
        q_out = dram.tile([NCORES * SLOTS, D], f32)

        # ---- layer-2 gather pre-staging --------------------------------
        gh_pool = ctx.enter_context(tc.tile_pool(name="gh", bufs=NCALL))
        gh_dma_sems = [nc.alloc_semaphore(f"gh_dma{k}") for k in range(NCALL)]
        gh_tiles = []

        def emit_preps(k1):
            while len(gh_tiles) < k1:
                k = len(gh_tiles)
                gt = gh_pool.tile([128, SUB, D], f8, tag="gh", name=f"gh{k}")
                gh_tiles.append(gt)
                nc.gpsimd.dma_gather(
                    gt, h1full_r[:],
                    idx_sb[:, k * SUB * 8:(k + 1) * SUB * 8],
                    SUB * 128, SUB * 128, D,
                    prepare_only=True, sem=gh_dma_sems[k],
                    queue_num=k % 2)

        emit_preps(min(PRE_AR1, NCALL))

        ident = const.tile([64, 64], f32)
        make_identity(nc, ident)

        def bn_vec(star, g_sb, be_sb, st):
            """star=[1,1024] (sum|sumsq) -> st=[1,1024] (scale|shift)."""
            a = vp.tile([1, D], f32, tag="bnv_a")
            b = vp.tile([1, D], f32, tag="bnv_b")
            c = vp.tile([1, D], f32, tag="bnv_c")
            nc.vector.tensor_scalar_mul(a, star[:, 0:D], 1.0 / N)
            nc.vector.tensor_scalar_mul(b, star[:, D:2 * D], 1.0 / N)
            nc.vector.tensor_mul(c, a, a)
            nc.vector.tensor_sub(b, b, c)                    # var
            nc.scalar.activation(c, b, AF.Sqrt, bias=eps1)   # sd
            nc.vector.reciprocal(b, c)                       # rstd
            nc.vector.tensor_mul(st[:, 0:D], b, g_sb)        # s
            nc.vector.tensor_mul(c, a, st[:, 0:D])           # mean*s
            nc.vector.tensor_sub(st[:, D:2 * D], be_sb, c)   # t
        def bn_broadcast(st, stb):
            """st=[1,1024] -> stb=[128,1024] via ones matmul."""
            sb_ps = psA.tile([128, D], f32, tag="work")
            tb_ps = psA.tile([128, D], f32, tag="work")
            nc.tensor.matmul(sb_ps, ones1, st[:, 0:D], start=True, stop=True)
            nc.tensor.matmul(tb_ps, ones1, st[:, D:2 * D], start=True, stop=True)
            nc.vector.tensor_copy(stb[:, 0:D], sb_ps)
            nc.vector.tensor_copy(stb[:, D:2 * D], tb_ps)

        # =================== Layer 1 ===================
        with ExitStack() as l1ctx:
            l1p = l1ctx.enter_context(tc.tile_pool(name="l1p", bufs=1))
            l1t = l1ctx.enter_context(tc.tile_pool(name="l1t", bufs=2))
            gx_pool = l1ctx.enter_context(tc.tile_pool(name="gx", bufs=3))

            zT = l1p.tile([KAUG, NPAD], f32)
            nc.sync.dma_start(out=zT, in_=xTaug)
            invb_sb = l1p.tile([DIN, NPAD], f32)
            nc.sync.dma_start(
                out=invb_sb,
                in_=bass.AP(tensor=invdegb.tensor, offset=invdegb.offset,
                            ap=[[0, DIN]] + list(invdegb.ap[1:])))

            for bidx in range(NBLK):
                Gxb = gx_pool.tile([128, T_BLK, XP], bf, tag="gx")
                nc.sync.dma_start(
                    out=Gxb, in_=Gx_d[:, bidx * T_BLK:(bidx + 1) * T_BLK, :])
                zt_ps = psA.tile([XP, 128], f32, tag="work")
                for t in range(T_BLK):
                    nc.tensor.matmul(zt_ps, Gxb[:, t, :],
                                     S_sb[:, bidx * T_BLK + t, :],
                                     start=(t == 0), stop=(t == T_BLK - 1))
                tmp = l1t.tile([DIN, 128], f32, tag="ztmp")
                nc.vector.tensor_mul(tmp, zt_ps[0:DIN, :],
                                     invb_sb[:, ts(bidx, 128)])
                nc.vector.tensor_add(zT[0:DIN, ts(bidx, 128)],
                                     zT[0:DIN, ts(bidx, 128)], tmp)

            u_sb = l1p.tile([128, NBLK, D], bf)
            sum_ps = psStat.tile([1, D], f32, tag="sum")
            ssq_ps = psStat.tile([1, D], f32, tag="ssq")
            for bidx in range(NBLK):
                u_ps = psA.tile([128, D], f32, tag="work")
                nc.tensor.matmul(u_ps, zT[:, ts(bidx, 128)], Waug_sb,
                                 start=True, stop=True)
                nc.vector.tensor_copy(u_sb[:, bidx, :], u_ps)
                usq = sq_pool.tile([128, D], bf, tag="usq")
                nc.scalar.square(usq, u_ps)
                nc.tensor.matmul(sum_ps, mask_bf[:, bidx:bidx + 1],
                                 u_sb[:, bidx, :],
                                 start=(bidx == 0), stop=(bidx == NBLK - 1))
                nc.tensor.matmul(ssq_ps, mask_bf[:, bidx:bidx + 1], usq,
                                 start=(bidx == 0), stop=(bidx == NBLK - 1))

            stats_sb = l1p.tile([1, 2 * D], f32)
            nc.vector.tensor_copy(stats_sb[:, 0:D], sum_ps)
            nc.vector.tensor_copy(stats_sb[:, D:2 * D], ssq_ps)
            nc.sync.dma_start(out=bnc_in[0][:], in_=stats_sb)
            nc.gpsimd.collective_compute(
                "AllReduce", ALU.add, replica_groups=RG,
                ins=[bnc_in[0].opt()], outs=[bnc_out[0].opt()])
            star1 = l1p.tile([1, 2 * D], f32)
            nc.sync.dma_start(out=star1, in_=bnc_out[0][:])

            emit_preps(min(PRE_AG, NCALL))

            st1 = l1p.tile([1, 2 * D], f32)
            bn_vec(star1, g1_sb, be1_sb, st1)
            stb1 = l1p.tile([128, 2 * D], f32)
            bn_broadcast(st1, stb1)

            for bidx in range(NBLK):
                t1 = l1t.tile([128, D], f32, tag="ap1")
                nc.vector.tensor_mul(t1, u_sb[:, bidx, :], stb1[:, 0:D])
                t2 = l1t.tile([128, D], f32, tag="ap2")
                nc.vector.tensor_add(t2, t1, stb1[:, D:2 * D])
                nc.scalar.activation(h1_sb[:, bidx, :], t2, AF.Relu)
                nc.scalar.activation(h1f8_sb[:, bidx, :], t2, AF.Relu)
                nb = min(128, NLOC - bidx * 128)
                nc.sync.dma_start(
                    out=h1loc[bidx * 128:bidx * 128 + nb, :],
                    in_=h1f8_sb[0:nb, bidx, :])

            nc.gpsimd.collective_compute(
                "AllGather", ALU.bypass, replica_groups=RG,
                ins=[h1loc.opt()], outs=[h1full.opt()])

        emit_preps(NCALL)
        # Fire the prepared gathers; the WAW edge on h1full orders the
        # triggers (and so the DMAs) after the AllGather.
        nc.gpsimd.trigger_dma(count=None, queue_num=0,
                              signals_writable=[h1full[0:1, 0:16]])
        nc.gpsimd.trigger_dma(count=None, queue_num=1,
                              signals_writable=[h1full[0:1, 0:16]])

        # =================== Layer 2 ===================
        l2ctx = ctx.enter_context(ExitStack())
        l2p = l2ctx.enter_context(tc.tile_pool(name="l2p", bufs=1))
        l2t = l2ctx.enter_context(tc.tile_pool(name="l2t", bufs=2))

        u2_sb = l2p.tile([128, NBLK, D], bf)
        sum2_ps = psStat.tile([1, D], f32, tag="sum")
        ssq2_ps = psStat.tile([1, D], f32, tag="ssq")
        for bidx in range(NBLK):
            agg_ps = psA.tile([128, D], f32, tag="work")
            for t in range(T_BLK):
                mm = nc.tensor.matmul(
                    agg_ps, S_sb[:, bidx * T_BLK + t, :],
                    gh_tiles[bidx * CPB + t // SUB][:, t % SUB, :],
                    start=(t == 0), stop=(t == T_BLK - 1))
                if t % SUB == 0:
                    mm._wait_ge(gh_dma_sems[bidx * CPB + t // SUB], 16)
            nc.vector.scalar_tensor_tensor(
                u2_sb[:, bidx, :], agg_ps, invpp_sb[:, bidx:bidx + 1],
                h1_sb[:, bidx, :], op0=ALU.mult, op1=ALU.add)
            usq2 = sq_pool.tile([128, D], bf, tag="usq")
            nc.scalar.square(usq2, u2_sb[:, bidx, :])
            nc.tensor.matmul(sum2_ps, mask_bf[:, bidx:bidx + 1],
                             u2_sb[:, bidx, :],
                             start=(bidx == 0), stop=(bidx == NBLK - 1))
            nc.tensor.matmul(ssq2_ps, mask_bf[:, bidx:bidx + 1], usq2,
                             start=(bidx == 0), stop=(bidx == NBLK - 1))

        sctx.close()

        stats2_sb = l2p.tile([1, 2 * D], f32)
        nc.vector.tensor_copy(stats2_sb[:, 0:D], sum2_ps)
        nc.vector.tensor_copy(stats2_sb[:, D:2 * D], ssq2_ps)
        nc.sync.dma_start(out=bnc_in[1][:], in_=stats2_sb)
        nc.gpsimd.collective_compute(
            "AllReduce", ALU.add, replica_groups=RG,
            ins=[bnc_in[1].opt()], outs=[bnc_out[1].opt()])
        star2 = l2p.tile([1, 2 * D], f32)
        nc.sync.dma_start(out=star2, in_=bnc_out[1][:])

        # ---- head constants + early head matmuls (overlap AllReduce) ----
        hp = ctx.enter_context(tc.tile_pool(name="hp", bufs=1, side="right"))
        hx = ctx.enter_context(tc.tile_pool(name="hx", bufs=2, side="right"))
        hv = ctx.enter_context(tc.tile_pool(name="hv", bufs=2, side="right"))
        W1_sb = hp.tile([128, 12, DH], bf)
        nc.sync.dma_start(out=W1_sb, in_=Wfc1t)
        W2_sb = hp.tile([128, 2, DH], bf)
        nc.sync.dma_start(out=W2_sb, in_=Wfc2t)
        W3_sb = hp.tile([128, 2, DH], bf)
        nc.sync.dma_start(out=W3_sb, in_=Wfc3t)
        W4_sb = hp.tile([128, 2, 1], bf)
        nc.sync.dma_start(out=W4_sb, in_=Wfc4t)
        gbT_sb = hp.tile([128, 2], f32)
        nc.sync.dma_start(out=gbT_sb, in_=gbT)
        bbT_sb = hp.tile([128, 2], f32)
        nc.sync.dma_start(out=bbT_sb, in_=bbT)
        gb2T_sb = hp.tile([128, 2], f32)
        nc.sync.dma_start(out=gb2T_sb, in_=gb2T)
        bb2T_sb = hp.tile([128, 2], f32)
        nc.sync.dma_start(out=bb2T_sb, in_=bb2T)
        gb3T_sb = hp.tile([128, 2], f32)
        nc.sync.dma_start(out=gb3T_sb, in_=gb3T)
        bb3T_sb = hp.tile([128, 2], f32)
        nc.sync.dma_start(out=bb3T_sb, in_=bb3T)
        b4_sb = hp.tile([128, 1], f32)
        nc.sync.dma_start(out=b4_sb, in_=b4rep)
        pgT_sb = hp.tile([128, 4, B], bf)
        nc.sync.dma_start(out=pgT_sb, in_=pgT)
        nghT_sb = hp.tile([128, 4, B * KCAND], bf)
        nc.sync.dma_start(out=nghT_sb, in_=neighT)

        def rep10(sl, nchunk):
            # [128, 64] slice -> [128, 320] with each column repeated 10x
            gstep = sl.ap[1][0]
            return bass.AP(tensor=sl.tensor, offset=sl.offset + nchunk * 32 * gstep,
                           ap=[list(sl.ap[0]), [gstep, 32], [0, 10]])

        HT_ps = [[psH.tile([128, 320], f32, name=f"ht{m}{n}", tag=f"hm{m}{n}")
                  for n in range(2)] for m in range(2)]
        for m in range(2):
            for n in range(2):
                for kt in range(4, 12):
                    if kt < 8:
                        rhs = rep10(pgT_sb[:, kt - 4, :], n)
                    else:
                        rhs = nghT_sb[:, kt - 8, n * 320:(n + 1) * 320]
                    nc.tensor.matmul(HT_ps[m][n],
                                     W1_sb[:, kt, ts(m, 128)], rhs,
                                     start=(kt == 4), stop=False)

        st2 = l2p.tile([1, 2 * D], f32)
        bn_vec(star2, g2_sb, be2_sb, st2)
        stb2 = l2p.tile([128, 2 * D], f32)
        bn_broadcast(st2, stb2)

        for bidx in range(NBLK):
            t1 = l2t.tile([128, D], f32, tag="ap1")
            nc.vector.tensor_mul(t1, u2_sb[:, bidx, :], stb2[:, 0:D])
            t2 = l2t.tile([128, D], f32, tag="ap2")
            nc.vector.tensor_add(t2, t1, stb2[:, D:2 * D])
            nc.scalar.activation(h2_sb[:, bidx, :], t2, AF.Relu)

        # =================== Readout ===================
        qs_ps = psStat.tile([SLOTS, D], f32, tag="sum")
        for bidx in range(NBLK):
            nc.tensor.matmul(qs_ps, C_sb[:, bidx, :], h2_sb[:, bidx, :],
                             start=(bidx == 0), stop=(bidx == NBLK - 1))
        qs_sb = l2p.tile([SLOTS, D], f32)
        nc.vector.tensor_copy(qs_sb, qs_ps)
        nc.sync.dma_start(out=q_in[:], in_=qs_sb)
        nc.gpsimd.collective_compute(
            "AllGather", ALU.bypass, replica_groups=RG,
            ins=[q_in.opt()], outs=[q_out.opt()])
        qg_sb = l2p.tile([NCORES * SLOTS, D], f32)
        nc.sync.dma_start(out=qg_sb, in_=q_out[:])
        qsum_ps = psStat.tile([B, D], f32, tag="ssq")
        nc.tensor.matmul(qsum_ps, Msel_sb, qg_sb, start=True, stop=True)
        qemb_sb = l2p.tile([B, D], f32)
        nc.scalar.activation(qemb_sb, qsum_ps, AF.Copy, scale=invcnt_sb)

        qT_sb = keep.tile([128, 4, B], bf)
        for j in range(4):
            qT_ps = psA.tile([128, B], f32, tag="work")
            nc.tensor.transpose(qT_ps, qemb_sb[:, ts(j, 128)], ident)
            nc.vector.tensor_copy(qT_sb[:, j, :], qT_ps)

        l2ctx.close()

        # =================== Head (bf16, feature-major) ===================
        # finish MM1 with the qemb k-tiles
        for m in range(2):
            for n in range(2):
                for kt in range(4):
                    rhs = rep10(qT_sb[:, kt, :], n)
                    nc.tensor.matmul(HT_ps[m][n],
                                     W1_sb[:, kt, ts(m, 128)], rhs,
                                     start=False, stop=(kt == 3))
        HT = hx.tile([128, 2, 640], f32, tag="hpre")
        for m in range(2):
            for n in range(2):
                nc.vector.tensor_copy(HT[:, m, n * 320:(n + 1) * 320],
                                      HT_ps[m][n])

        def head_bn_relu(pre_sb, gT, bT_, out_sb):
            """pre_sb [128, 2, 640] f32; BN over 640 rows + ReLU -> bf16 out_sb."""
            for m in range(2):
                cat = pre_sb[:, m, :]
                sums = hv.tile([128, 1], f32, tag="hsum")
                nc.vector.tensor_reduce(sums, cat, mybir.AxisListType.X,
                                        ALU.add)
                sqj = hv.tile([128, 640], f32, tag="hsq")
                ssq = hv.tile([128, 1], f32, tag="hssq")
                nc.scalar.activation(sqj, cat, AF.Square, accum_out=ssq)
                mean = hv.tile([128, 1], f32, tag="hmean")
                nc.vector.tensor_scalar_mul(mean, sums, 1.0 / 640.0)
                ex2 = hv.tile([128, 1], f32, tag="hex2")
                nc.vector.tensor_scalar_mul(ex2, ssq, 1.0 / 640.0)
                msq = hv.tile([128, 1], f32, tag="hmsq")
                nc.vector.tensor_mul(msq, mean, mean)
                var = hv.tile([128, 1], f32, tag="hvar")
                nc.vector.tensor_sub(var, ex2, msq)
                sd = hv.tile([128, 1], f32, tag="hsd")
                nc.scalar.activation(sd, var, AF.Sqrt, bias=eps128)
                rstd = hv.tile([128, 1], f32, tag="hrstd")
                nc.vector.reciprocal(rstd, sd)
                s = hv.tile([128, 1], f32, tag="hs")
                nc.vector.tensor_mul(s, rstd, gT[:, m:m + 1])
                ms = hv.tile([128, 1], f32, tag="hms")
                nc.vector.tensor_mul(ms, mean, s)
                t = hv.tile([128, 1], f32, tag="ht")
                nc.vector.tensor_sub(t, bT_[:, m:m + 1], ms)
                nc.scalar.activation(out_sb[:, m, :], cat, AF.Relu,
                                     scale=s, bias=t)

        def head_layer_mm(rhs_in, W_sb, pre_sb):
            for m in range(2):
                for n in range(2):
                    ps = psH.tile([128, 320], f32, tag=f"hm{m}{n}")
                    for kt in range(2):
                        nc.tensor.matmul(ps, W_sb[:, kt, ts(m, 128)],
                                         rhs_in[:, kt, n * 320:(n + 1) * 320],
                                         start=(kt == 0), stop=(kt == 1))
                    nc.vector.tensor_copy(pre_sb[:, m, n * 320:(n + 1) * 320],
                                          ps)

        H1h = hx.tile([128, 2, 640], bf, tag="hact")
        head_bn_relu(HT, gbT_sb, bbT_sb, H1h)

        H2p = hx.tile([128, 2, 640], f32, tag="hpre")
        head_layer_mm(H1h, W2_sb, H2p)
        H2h = hx.tile([128, 2, 640], bf, tag="hact")
        head_bn_relu(H2p, gb2T_sb, bb2T_sb, H2h)

        H3p = hx.tile([128, 2, 640], f32, tag="hpre")
        head_layer_mm(H2h, W3_sb, H3p)
        H3h = hx.tile([128, 2, 640], bf, tag="hact")
        head_bn_relu(H3p, gb3T_sb, bb3T_sb, H3h)

        pred_sb = hp.tile([128, 5], f32)
        for rr in range(5):
            pr_ps = psA.tile([128, 1], f32, tag="work")
            for kt in range(2):
                nc.tensor.matmul(pr_ps, H3h[:, kt, ts(rr, 128)],
                                 W4_sb[:, kt, :],
                                 start=(kt == 0), stop=(kt == 1))
            nc.scalar.activation(pred_sb[:, rr:rr + 1], pr_ps, AF.Sigmoid,
                                 bias=b4_sb)

        nc.sync.dma_start(
            out=bass.AP(tensor=outp.tensor, offset=outp.offset,
                        ap=[[1, 128], [128, 5]]),
            in_=pred_sb)
    nc.compile()
    return nc


# ---------------------------------------------------------------------------
# Entry point
# ---------------------------------------------------------------------------

def kernel(**inputs) -> np.ndarray:
    global LAST_EXEC_NS
    from concourse.bass_utils import run_bass_kernel_spmd

    in_maps, T_BLK, SLOTS = preprocess(**inputs)
    nc = build_nc(T_BLK, SLOTS)

    trace = bool(int(os.environ.get("GNN_TRACE", "0")))
    kw = {}
    if trace:
        kw = dict(trace=True, trace_cores=list(range(NCORES)),
                  stitch_traces=False)
    try:
        res = run_bass_kernel_spmd(nc, in_maps, core_ids=list(range(NCORES)),
                                   **kw)
    except Exception:
        if not trace:
            raise
        res = run_bass_kernel_spmd(nc, in_maps, core_ids=list(range(NCORES)))
    LAST_EXEC_NS = res.exec_time_ns
    return np.asarray(res.results[0]["outp"], np.float32)


# revision 20
# speedup vs baseline: 1.5446x; 1.0399x over previous
"""Trainium2 Bass kernel for nn_Model_40827959116312 (GIN message passing + MLP head).

Self-contained: builds per-core graph structures on host (numpy), compiles a
Bass/Tile SPMD program for 8 NeuronCores, runs via run_bass_kernel_spmd, and
returns the full [64, 10] output.

Sharding: data-parallel over destination nodes (1250 per core, 10 blocks of
128). Layer-1 aggregation runs on host-pregathered x rows (the projection
commutes with the mean-aggregation, so only the 21-wide augmented features are
aggregated). Layer-2 gathers fp8 h1 rows via dma_gather with per-block
source deduplication. All gather descriptor generation (the serial ~8.5us/call
Q7 cost) is pre-staged with prepare_only during layer 1 + the collectives,
reading h1full through an address-alias tensor so Tile doesn't pin the
AllGather dependency on the preps; a trigger_dma ordered after the AllGather
(WAW via signals_writable) fires the DMAs, and consumers wait per-call DMA
semaphores. Segment-sums are one-hot/count matmuls accumulating in PSUM
(count matrices in fp8 — exact for small integer counts). BatchNorm
statistics are [2,512] AllReduces; h1 is AllGathered in fp8. The per-graph
readout uses a compact per-core slot layout + small AllGather + combine
matmul instead of a [64,512] AllReduce. The MLP head is replicated on every
core (feature-major bf16, fused BN+ReLU activations)."""

import os
import numpy as np
import ml_dtypes

bf16 = ml_dtypes.bfloat16
f8e4 = ml_dtypes.float8_e4m3

# Problem constants (from spec).
N, E, B, KCAND = 10000, 160000, 64, 10
DIN, D, DH = 20, 512, 256
NCORES = 8
NLOC = N // NCORES            # 1250
NBLK = (NLOC + 127) // 128    # 10
NPAD = NBLK * 128             # 1280
XP = 32                       # padded x feature width (host-pregathered)
KAUG = DIN + 1                # 21 (features + bias row)
BN_EPS = 1e-5
SUB = 8                       # gather sub-call size: SUB*128 = 1024 indices
PRE_AR1 = 12                  # preps emitted before the BN1-stats AllReduce
PRE_AG = 16                  # preps emitted before the h1 AllGather dispatch
PRE_T1 = 99                   # preps before the first trigger (>=NCALL: one)

LAST_EXEC_NS = None           # set by kernel() when profiling succeeds


# ---------------------------------------------------------------------------
# Host-side preprocessing
# ---------------------------------------------------------------------------

def preprocess(x, pg_emb, neigh_emb, W_init, b_init, g1, be1, g2, be2,
               W_fc, b_fc, W_fc2, b_fc2, W_fc3, b_fc3, W_fc4, b_fc4,
               gb, bb, gb2, bb2, gb3, bb3, edge_src, edge_dst, node2graph):
    """Build per-core input maps + the uniform per-block tile count T_BLK."""
    x = np.asarray(x, np.float32)
    edge_src = np.asarray(edge_src, np.int64)
    edge_dst = np.asarray(edge_dst, np.int64)
    node2graph = np.asarray(node2graph, np.int64)

    deg = np.bincount(edge_dst, minlength=N).astype(np.float64)
    invdeg = (1.0 / np.maximum(deg, 1.0)).astype(np.float32)
    r = (deg > 0).astype(np.float32)

    # Per (core, block): unique sources + count matrices.
    per_core = []
    t_blk = SUB
    for c in range(NCORES):
        lo = c * NLOC
        sel = (edge_dst >= lo) & (edge_dst < lo + NLOC)
        s_c = edge_src[sel]
        d_c = edge_dst[sel] - lo
        blocks = []
        for bidx in range(NBLK):
            bsel = (d_c >> 7) == bidx
            s_b = s_c[bsel]
            d_b = d_c[bsel] - (bidx << 7)
            uniq, inv = np.unique(s_b, return_inverse=True)
            blocks.append((uniq, inv, d_b))
            t_blk = max(t_blk, (len(uniq) + 127) // 128)
        per_core.append(blocks)

    T_BLK = ((t_blk + SUB - 1) // SUB) * SUB   # multiple of SUB
    NT = NBLK * T_BLK              # total edge tiles per core
    NU = T_BLK * 128               # padded unique srcs per block

    xpad = np.zeros((N, XP), np.float32)
    xpad[:, :DIN] = x
    xpad_bf = xpad.astype(bf16)

    W_aug = np.zeros((KAUG, D), np.float32)
    W_aug[:DIN] = np.asarray(W_init, np.float32)
    W_aug[DIN] = np.asarray(b_init, np.float32)

    cnt = np.bincount(node2graph, minlength=B).astype(np.float64)
    inv_cnt = (1.0 / np.maximum(cnt, 1.0)).astype(np.float32).reshape(B, 1)

    # Compact per-core readout slots: core c's nodes span graphs
    # [g_lo[c], g_hi[c]]; SLOTS = max span so the combine matrix is uniform.
    g_lo = [int(node2graph[c * NLOC]) for c in range(NCORES)]
    g_hi = [int(node2graph[(c + 1) * NLOC - 1]) for c in range(NCORES)]
    SLOTS = max(h - l + 1 for l, h in zip(g_lo, g_hi))
    Msel = np.zeros((NCORES * SLOTS, B), np.float32)
    for c in range(NCORES):
        for j in range(SLOTS):
            g = g_lo[c] + j
            if g <= g_hi[c] and g < B:
                Msel[c * SLOTS + j, g] = 1.0

    # Head weights, feature-major bf16 layouts.
    W_fc = np.asarray(W_fc, np.float32)      # [1536, 256]
    Wfc1t = np.ascontiguousarray(
        W_fc.reshape(12, 128, DH).transpose(1, 0, 2)).astype(bf16)
    Wfc2t = np.ascontiguousarray(
        np.asarray(W_fc2, np.float32).reshape(2, 128, DH).transpose(1, 0, 2)).astype(bf16)
    Wfc3t = np.ascontiguousarray(
        np.asarray(W_fc3, np.float32).reshape(2, 128, DH).transpose(1, 0, 2)).astype(bf16)
    Wfc4t = np.ascontiguousarray(
        np.asarray(W_fc4, np.float32).reshape(2, 128, 1).transpose(1, 0, 2)).astype(bf16)

    def ppart(v):  # [256] -> [128, 2] (dh = kt*128 + p)
        return np.ascontiguousarray(np.asarray(v, np.float32).reshape(2, 128).T)

    pgT = np.ascontiguousarray(
        np.asarray(pg_emb, np.float32).T.reshape(4, 128, B).transpose(1, 0, 2)).astype(bf16)
    neighT = np.ascontiguousarray(
        np.asarray(neigh_emb, np.float32).reshape(B * KCAND, D).T
        .reshape(4, 128, B * KCAND).transpose(1, 0, 2)).astype(bf16)
    b4rep = np.full((128, 1), float(np.asarray(b_fc4).reshape(-1)[0]), np.float32)

    shared = dict(
        W_aug=W_aug,
        bn1g=np.asarray(g1, np.float32).reshape(1, D),
        bn1b=np.asarray(be1, np.float32).reshape(1, D),
        bn2g=np.asarray(g2, np.float32).reshape(1, D),
        bn2b=np.asarray(be2, np.float32).reshape(1, D),
        invcnt=inv_cnt,
        Msel=Msel,
        Wfc1t=Wfc1t, Wfc2t=Wfc2t, Wfc3t=Wfc3t, Wfc4t=Wfc4t,
        gbT=ppart(gb), bbT=ppart(bb),
        gb2T=ppart(gb2), bb2T=ppart(bb2),
        gb3T=ppart(gb3), bb3T=ppart(bb3),
        b4rep=b4rep,
        pgT=pgT, neighT=neighT,
    )

    in_maps = []
    for c in range(NCORES):
        lo = c * NLOC
        S = np.zeros((NT, 128, 128), np.float32)
        idx_flat = np.zeros(NT * 128, np.int64)
        for bidx in range(NBLK):
            uniq, inv, d_b = per_core[c][bidx]
            Sb = np.zeros((NU, 128), np.float32)
            np.add.at(Sb, (inv, d_b), 1.0)
            S[bidx * T_BLK:(bidx + 1) * T_BLK] = Sb.reshape(T_BLK, 128, 128)
            idx_flat[bidx * NU: bidx * NU + len(uniq)] = uniq
        # wrap: slot i lives at [i % 16, i // 16], tiled over 128 partitions
        idx_w = np.tile(idx_flat.reshape(-1, 16).T, (8, 1)).astype(np.int16)
        # host-pregathered x rows in the device SBUF layout [128, NT, XP]
        Gx = np.ascontiguousarray(xpad_bf[idx_flat.reshape(NT, 128).T])

        nloc_ids = np.arange(NPAD)
        real = nloc_ids < NLOC
        gids = np.minimum(lo + nloc_ids, N - 1)

        xTaug = np.zeros((KAUG, NPAD), np.float32)
        xTaug[:DIN, :NLOC] = x[lo:lo + NLOC].T
        xTaug[DIN, :NLOC] = 1.0 + r[lo:lo + NLOC]

        invdegb = np.zeros((1, NPAD), np.float32)
        invdegb[0, :NLOC] = invdeg[lo:lo + NLOC]

        invdeg_pp = np.where(real, invdeg[gids], 0.0).reshape(NBLK, 128).T.copy()
        maskn = real.astype(np.float32).reshape(NBLK, 128).T.copy()

        # per-graph readout into SLOTS compact rows (graph g -> g - g_lo[c])
        Cro = np.zeros((NBLK, 128, SLOTS), np.float32)
        n2g_loc = node2graph[lo:lo + NLOC]
        Cro.reshape(NPAD, SLOTS)[nloc_ids[real], n2g_loc - g_lo[c]] = 1.0
        Cro = np.ascontiguousarray(Cro.transpose(1, 0, 2))

        m = dict(shared)
        m.update(
            idx=idx_w,
            S=np.ascontiguousarray(S.transpose(1, 0, 2)).astype(f8e4),
            Gx=Gx,
            xTaug=xTaug,
            invdegb=invdegb,
            invdeg_pp=invdeg_pp,
            maskn=maskn,
            Cro=Cro,
        )
        in_maps.append(m)

    return in_maps, T_BLK, SLOTS


# ---------------------------------------------------------------------------
# Device program
# ---------------------------------------------------------------------------

def build_nc(T_BLK, SLOTS):
    from contextlib import ExitStack

    import concourse.bass as bass
    import concourse.mybir as mybir
    import concourse.tile as tile
    from concourse import bacc
    from concourse.bass import ts
    from concourse.masks import make_identity

    f32 = mybir.dt.float32
    bf = mybir.dt.bfloat16
    f8 = mybir.dt.float8e4
    i16 = mybir.dt.int16
    AF = mybir.ActivationFunctionType
    ALU = mybir.AluOpType

    NT = NBLK * T_BLK
    NCALL = NT // SUB              # layer-2 gather sub-calls
    CPB = T_BLK // SUB             # sub-calls per block
    RG = [list(range(NCORES))]

    nc = bacc.Bacc("TRN2", target_bir_lowering=False, debug=False,
                   num_devices=NCORES, num_swdge_queues=4)

    def din(name, shape, dt):
        return nc.dram_tensor(name, list(shape), dt, kind="ExternalInput").ap()

    idx = din("idx", (128, NT * 8), i16)
    S = din("S", (128, NT, 128), f8)
    Gx_d = din("Gx", (128, NT, XP), bf)
    xTaug = din("xTaug", (KAUG, NPAD), f32)
    invdegb = din("invdegb", (1, NPAD), f32)
    invdeg_pp = din("invdeg_pp", (128, NBLK), f32)
    maskn = din("maskn", (128, NBLK), f32)
    W_aug = din("W_aug", (KAUG, D), f32)
    bn1g = din("bn1g", (1, D), f32)
    bn1b = din("bn1b", (1, D), f32)
    bn2g = din("bn2g", (1, D), f32)
    bn2b = din("bn2b", (1, D), f32)
    Cro = din("Cro", (128, NBLK, SLOTS), f32)
    Msel = din("Msel", (NCORES * SLOTS, B), f32)
    invcnt = din("invcnt", (B, 1), f32)
    Wfc1t = din("Wfc1t", (128, 12, DH), bf)
    Wfc2t = din("Wfc2t", (128, 2, DH), bf)
    Wfc3t = din("Wfc3t", (128, 2, DH), bf)
    Wfc4t = din("Wfc4t", (128, 2, 1), bf)
    gbT = din("gbT", (128, 2), f32)
    bbT = din("bbT", (128, 2), f32)
    gb2T = din("gb2T", (128, 2), f32)
    bb2T = din("bb2T", (128, 2), f32)
    gb3T = din("gb3T", (128, 2), f32)
    bb3T = din("bb3T", (128, 2), f32)
    b4rep = din("b4rep", (128, 1), f32)
    pgT = din("pgT", (128, 4, B), bf)
    neighT = din("neighT", (128, 4, B * KCAND), bf)

    outp = nc.dram_tensor("outp", [B, KCAND], f32, kind="ExternalOutput").ap()

    # h1full: AllGather output. The gather preps read it through an address
    # alias ("h1full_r") so Tile doesn't order the descriptor generation
    # after the AllGather; the trigger is ordered after the AllGather via a
    # WAW edge (signals_writable) instead.
    h1full_h = nc.dram_tensor("h1full", [N, D], f8, kind="Internal",
                              addr_space="Shared")
    h1full = h1full_h.ap()
    _mloc = nc.lookup_mloc(h1full_h)
    _amls = nc._tensor("h1full_r", [N, D], f8, kind="Internal", type="DRAM",
                       addr_space="Shared")
    _amls.memorylocations[0].addr = _mloc.addr
    _amls.memorylocations[0].allocated = _mloc.allocated
    h1full_r = bass.DRamTensorHandle("h1full_r", [N, D], f8).ap()

    with tile.TileContext(nc) as tc, ExitStack() as ctx:
        const = ctx.enter_context(tc.tile_pool(name="const", bufs=1))
        dram = ctx.enter_context(tc.tile_pool(name="dram", bufs=1, space="DRAM"))
        # PSUM static budget (8 banks): work(2) + sum(1) + ssq(1) + hm(4).
        psA = ctx.enter_context(tc.tile_pool(name="psA", bufs=2, space="PSUM"))
        psStat = ctx.enter_context(tc.tile_pool(name="psStat", bufs=1, space="PSUM"))
        psH = ctx.enter_context(tc.tile_pool(name="psH", bufs=1, space="PSUM"))
        vp = ctx.enter_context(tc.tile_pool(name="vp", bufs=1))
        sq_pool = ctx.enter_context(tc.tile_pool(name="sq", bufs=2))
        keep = ctx.enter_context(tc.tile_pool(name="keep", bufs=1))

        # ---- collective warmup (absorbs first-collective setup + skew) ----
        wu_in = dram.tile([1, 16], f32)
        wu_out = dram.tile([1, 16], f32)
        wu_sb = const.tile([1, 16], f32)
        nc.vector.memset(wu_sb, 1.0)
        nc.sync.dma_start(out=wu_in[:], in_=wu_sb)
        nc.gpsimd.collective_compute(
            "AllReduce", ALU.add, replica_groups=RG,
            ins=[wu_in.opt()], outs=[wu_out.opt()])

        # ---- resident loads (idx first: the gather preps depend on it) ----
        sctx = ctx.enter_context(ExitStack())
        spool = sctx.enter_context(tc.tile_pool(name="spool", bufs=1, side="right"))
        idx_sb = spool.tile([128, NT * 8], i16)
        nc.sync.dma_start(out=idx_sb, in_=idx)
        S_sb = spool.tile([128, NT, 128], f8)
        for bidx in range(NBLK):
            nc.sync.dma_start(
                out=S_sb[:, bidx * T_BLK:(bidx + 1) * T_BLK, :],
                in_=S[:, bidx * T_BLK:(bidx + 1) * T_BLK, :])
        Waug_sb = const.tile([KAUG, D], f32)
        nc.sync.dma_start(out=Waug_sb, in_=W_aug)
        invpp_sb = const.tile([128, NBLK], f32)
        nc.sync.dma_start(out=invpp_sb, in_=invdeg_pp)
        mask_sb = const.tile([128, NBLK], f32)
        nc.sync.dma_start(out=mask_sb, in_=maskn)
        mask_bf = const.tile([128, NBLK], bf)
        nc.vector.tensor_copy(mask_bf, mask_sb)
        g1_sb = const.tile([1, D], f32)
        nc.sync.dma_start(out=g1_sb, in_=bn1g)
        be1_sb = const.tile([1, D], f32)
        nc.sync.dma_start(out=be1_sb, in_=bn1b)
        g2_sb = const.tile([1, D], f32)
        nc.sync.dma_start(out=g2_sb, in_=bn2g)
        be2_sb = const.tile([1, D], f32)
        nc.sync.dma_start(out=be2_sb, in_=bn2b)
        C_f32 = const.tile([128, NBLK, SLOTS], f32)
        nc.sync.dma_start(out=C_f32, in_=Cro)
        C_sb = const.tile([128, NBLK, SLOTS], bf)
        nc.vector.tensor_copy(C_sb, C_f32)
        Msel_sb = const.tile([NCORES * SLOTS, B], f32)
        nc.sync.dma_start(out=Msel_sb, in_=Msel)
        invcnt_sb = const.tile([B, 1], f32)
        nc.sync.dma_start(out=invcnt_sb, in_=invcnt)

        ones1 = const.tile([1, 128], f32)
        nc.vector.memset(ones1, 1.0)
        eps1 = const.tile([1, 1], f32)
        nc.vector.memset(eps1, BN_EPS)
        eps128 = const.tile([128, 1], f32)
        nc.vector.memset(eps128, BN_EPS)

        h1_sb = keep.tile([128, NBLK, D], bf)      # bf16 h1 (local rows)
        h1f8_sb = keep.tile([128, NBLK, D], f8)    # fp8 h1 (for the AllGather)
        h2_sb = keep.tile([128, NBLK, D], bf)      # bf16 h2 (local rows)

        h1loc = dram.tile([NLOC, D], f8)
        bnc_in = [dram.tile([1, 2 * D], f32, name=f"bi{i}") for i in range(2)]
        bnc_out = [dram.tile([1, 2 * D], f32, name=f"bo{i}") for i in range(2)]
        q_in = dram.tile([SLOTS, D], f32)
        q_out = dram.tile([NCORES * SLOTS, D], f32)

        # ---- layer-2 gather pre-staging --------------------------------
        gh_pool = ctx.enter_context(tc.tile_pool(name="gh", bufs=NCALL))
        gh_dma_sems = [nc.alloc_semaphore(f"gh_dma{k}") for k in range(NCALL)]
        gh_tiles = []

        def emit_preps(k1):
            while len(gh_tiles) < k1:
                k = len(gh_tiles)
                gt = gh_pool.tile([128, SUB, D], f8, tag="gh", name=f"gh{k}")
                gh_tiles.append(gt)
                nc.gpsimd.dma_gather(
                    gt, h1full_r[:],
                    idx_sb[:, k * SUB * 8:(k + 1) * SUB * 8],
                    SUB * 128, SUB * 128, D,
                    prepare_only=True, sem=gh_dma_sems[k],
                    queue_num=k % 4)

        emit_preps(min(PRE_AR1, NCALL))

        ident = const.tile([64, 64], f32)
        make_identity(nc, ident)

        def bn_vec(star, g_sb, be_sb, st):
            """star=[1,1024] (sum|sumsq) -> st=[1,1024] (scale|shift)."""
            a = vp.tile([1, D], f32, tag="bnv_a")
            b = vp.tile([1, D], f32, tag="bnv_b")
            c = vp.tile([1, D], f32, tag="bnv_c")
            nc.vector.tensor_scalar_mul(a, star[:, 0:D], 1.0 / N)
            nc.vector.tensor_scalar_mul(b, star[:, D:2 * D], 1.0 / N)
            nc.vector.tensor_mul(c, a, a)
            nc.vector.tensor_sub(b, b, c)                    # var
            nc.scalar.activation(c, b, AF.Sqrt, bias=eps1)   # sd
            nc.vector.reciprocal(b, c)                       # rstd
            nc.vector.tensor_mul(st[:, 0:D], b, g_sb)        # s
            nc.vector.tensor_mul(c, a, st[:, 0:D])           # mean*s
            nc.vector.tensor_sub(st[:, D:2 * D], be_sb, c)   # t
        def bn_broadcast(st, stb):
            """st=[1,1024] -> stb=[128,1024] via ones matmul."""
            sb_ps = psA.tile([128, D], f32, tag="work")
            tb_ps = psA.tile([128, D], f32, tag="work")
            nc.tensor.matmul(sb_ps, ones1, st[:, 0:D], start=True, stop=True)
            nc.tensor.matmul(tb_ps, ones1, st[:, D:2 * D], start=True, stop=True)
            nc.vector.tensor_copy(stb[:, 0:D], sb_ps)
            nc.vector.tensor_copy(stb[:, D:2 * D], tb_ps)

        # =================== Layer 1 ===================
        with ExitStack() as l1ctx:
            l1p = l1ctx.enter_context(tc.tile_pool(name="l1p", bufs=1))
            l1t = l1ctx.enter_context(tc.tile_pool(name="l1t", bufs=2))
            gx_pool = l1ctx.enter_context(tc.tile_pool(name="gx", bufs=3))

            zT = l1p.tile([KAUG, NPAD], f32)
            nc.sync.dma_start(out=zT, in_=xTaug)
            invb_sb = l1p.tile([DIN, NPAD], f32)
            nc.sync.dma_start(
                out=invb_sb,
                in_=bass.AP(tensor=invdegb.tensor, offset=invdegb.offset,
                            ap=[[0, DIN]] + list(invdegb.ap[1:])))

            for bidx in range(NBLK):
                Gxb = gx_pool.tile([128, T_BLK, XP], bf, tag="gx")
                nc.sync.dma_start(
                    out=Gxb, in_=Gx_d[:, bidx * T_BLK:(bidx + 1) * T_BLK, :])
                zt_ps = psA.tile([XP, 128], f32, tag="work")
                for t in range(T_BLK):
                    nc.tensor.matmul(zt_ps, Gxb[:, t, :],
                                     S_sb[:, bidx * T_BLK + t, :],
                                     start=(t == 0), stop=(t == T_BLK - 1))
                tmp = l1t.tile([DIN, 128], f32, tag="ztmp")
                nc.vector.tensor_mul(tmp, zt_ps[0:DIN, :],
                                     invb_sb[:, ts(bidx, 128)])
                nc.vector.tensor_add(zT[0:DIN, ts(bidx, 128)],
                                     zT[0:DIN, ts(bidx, 128)], tmp)

            u_sb = l1p.tile([128, NBLK, D], bf)
            sum_ps = psStat.tile([1, D], f32, tag="sum")
            ssq_ps = psStat.tile([1, D], f32, tag="ssq")
            for bidx in range(NBLK):
                u_ps = psA.tile([128, D], f32, tag="work")
                nc.tensor.matmul(u_ps, zT[:, ts(bidx, 128)], Waug_sb,
                                 start=True, stop=True)
                nc.vector.tensor_copy(u_sb[:, bidx, :], u_ps)
                usq = sq_pool.tile([128, D], bf, tag="usq")
                nc.scalar.square(usq, u_ps)
                nc.tensor.matmul(sum_ps, mask_bf[:, bidx:bidx + 1],
                                 u_sb[:, bidx, :],
                                 start=(bidx == 0), stop=(bidx == NBLK - 1))
                nc.tensor.matmul(ssq_ps, mask_bf[:, bidx:bidx + 1], usq,
                                 start=(bidx == 0), stop=(bidx == NBLK - 1))

            stats_sb = l1p.tile([1, 2 * D], f32)
            nc.vector.tensor_copy(stats_sb[:, 0:D], sum_ps)
            nc.vector.tensor_copy(stats_sb[:, D:2 * D], ssq_ps)
            nc.sync.dma_start(out=bnc_in[0][:], in_=stats_sb)
            nc.gpsimd.collective_compute(
                "AllReduce", ALU.add, replica_groups=RG,
                ins=[bnc_in[0].opt()], outs=[bnc_out[0].opt()])
            star1 = l1p.tile([1, 2 * D], f32)
            nc.sync.dma_start(out=star1, in_=bnc_out[0][:])

            emit_preps(min(PRE_AG, NCALL))

            st1 = l1p.tile([1, 2 * D], f32)
            bn_vec(star1, g1_sb, be1_sb, st1)
            stb1 = l1p.tile([128, 2 * D], f32)
            bn_broadcast(st1, stb1)

            for bidx in range(NBLK):
                t1 = l1t.tile([128, D], f32, tag="ap1")
                nc.vector.tensor_mul(t1, u_sb[:, bidx, :], stb1[:, 0:D])
                t2 = l1t.tile([128, D], f32, tag="ap2")
                nc.vector.tensor_add(t2, t1, stb1[:, D:2 * D])
                nc.scalar.activation(h1_sb[:, bidx, :], t2, AF.Relu)
                nc.scalar.activation(h1f8_sb[:, bidx, :], t2, AF.Relu)
                nb = min(128, NLOC - bidx * 128)
                nc.sync.dma_start(
                    out=h1loc[bidx * 128:bidx * 128 + nb, :],
                    in_=h1f8_sb[0:nb, bidx, :])

            nc.gpsimd.collective_compute(
                "AllGather", ALU.bypass, replica_groups=RG,
                ins=[h1loc.opt()], outs=[h1full.opt()])

        emit_preps(NCALL)
        # Fire the prepared gathers; the WAW edge on h1full orders the
        # triggers (and so the DMAs) after the AllGather.
        for q in range(4):
            nc.gpsimd.trigger_dma(
                count=None, queue_num=q,
                signals_writable=[h1full[0:1, 0:16]] +
                [gh_tiles[k][:] for k in range(q, NCALL, 4)])

        # =================== Layer 2 ===================
        l2ctx = ctx.enter_context(ExitStack())
        l2p = l2ctx.enter_context(tc.tile_pool(name="l2p", bufs=1))
        l2t = l2ctx.enter_context(tc.tile_pool(name="l2t", bufs=2))

        u2_sb = l2p.tile([128, NBLK, D], bf)
        sum2_ps = psStat.tile([1, D], f32, tag="sum")
        ssq2_ps = psStat.tile([1, D], f32, tag="ssq")
        for bidx in range(NBLK):
            agg_ps = psA.tile([128, D], f32, tag="work")
            for t in range(0, T_BLK, 2):
                mm = nc.tensor.matmul(
                    agg_ps, S_sb[:, bidx * T_BLK + t:bidx * T_BLK + t + 2, :],
                    gh_tiles[bidx * CPB + t // SUB][:, t % SUB:t % SUB + 2, :],
                    start=(t == 0), stop=(t == T_BLK - 2),
                    perf_mode=mybir.MatmulPerfMode.DoubleRow)
                if t % SUB == 0:
                    mm._wait_ge(gh_dma_sems[bidx * CPB + t // SUB], 16)
            nc.vector.scalar_tensor_tensor(
                u2_sb[:, bidx, :], agg_ps, invpp_sb[:, bidx:bidx + 1],
                h1_sb[:, bidx, :], op0=ALU.mult, op1=ALU.add)
            usq2 = sq_pool.tile([128, D], bf, tag="usq")
            nc.scalar.square(usq2, u2_sb[:, bidx, :])
            nc.tensor.matmul(sum2_ps, mask_bf[:, bidx:bidx + 1],
                             u2_sb[:, bidx, :],
                             start=(bidx == 0), stop=(bidx == NBLK - 1))
            nc.tensor.matmul(ssq2_ps, mask_bf[:, bidx:bidx + 1], usq2,
                             start=(bidx == 0), stop=(bidx == NBLK - 1))

        sctx.close()

        stats2_sb = l2p.tile([1, 2 * D], f32)
        nc.vector.tensor_copy(stats2_sb[:, 0:D], sum2_ps)
        nc.vector.tensor_copy(stats2_sb[:, D:2 * D], ssq2_ps)
        nc.sync.dma_start(out=bnc_in[1][:], in_=stats2_sb)
        nc.gpsimd.collective_compute(
            "AllReduce", ALU.add, replica_groups=RG,
            ins=[bnc_in[1].opt()], outs=[bnc_out[1].opt()])
        star2 = l2p.tile([1, 2 * D], f32)
        nc.sync.dma_start(out=star2, in_=bnc_out[1][:])

        # ---- head constants + early head matmuls (overlap AllReduce) ----
        hp = ctx.enter_context(tc.tile_pool(name="hp", bufs=1, side="right"))
        hx = ctx.enter_context(tc.tile_pool(name="hx", bufs=2, side="right"))
        hv = ctx.enter_context(tc.tile_pool(name="hv", bufs=2, side="right"))
        W1_sb = hp.tile([128, 12, DH], bf)
        nc.sync.dma_start(out=W1_sb, in_=Wfc1t)
        W2_sb = hp.tile([128, 2, DH], bf)
        nc.sync.dma_start(out=W2_sb, in_=Wfc2t)
        W3_sb = hp.tile([128, 2, DH], bf)
        nc.sync.dma_start(out=W3_sb, in_=Wfc3t)
        W4_sb = hp.tile([128, 2, 1], bf)
        nc.sync.dma_start(out=W4_sb, in_=Wfc4t)
        gbT_sb = hp.tile([128, 2], f32)
        nc.sync.dma_start(out=gbT_sb, in_=gbT)
        bbT_sb = hp.tile([128, 2], f32)
        nc.sync.dma_start(out=bbT_sb, in_=bbT)
        gb2T_sb = hp.tile([128, 2], f32)
        nc.sync.dma_start(out=gb2T_sb, in_=gb2T)
        bb2T_sb = hp.tile([128, 2], f32)
        nc.sync.dma_start(out=bb2T_sb, in_=bb2T)
        gb3T_sb = hp.tile([128, 2], f32)
        nc.sync.dma_start(out=gb3T_sb, in_=gb3T)
        bb3T_sb = hp.tile([128, 2], f32)
        nc.sync.dma_start(out=bb3T_sb, in_=bb3T)
        b4_sb = hp.tile([128, 1], f32)
        nc.sync.dma_start(out=b4_sb, in_=b4rep)
        pgT_sb = hp.tile([128, 4, B], bf)
        nc.sync.dma_start(out=pgT_sb, in_=pgT)
        nghT_sb = hp.tile([128, 4, B * KCAND], bf)
        nc.sync.dma_start(out=nghT_sb, in_=neighT)

        def rep10(sl, nchunk):
            # [128, 64] slice -> [128, 320] with each column repeated 10x
            gstep = sl.ap[1][0]
            return bass.AP(tensor=sl.tensor, offset=sl.offset + nchunk * 32 * gstep,
                           ap=[list(sl.ap[0]), [gstep, 32], [0, 10]])

        HT_ps = [[psH.tile([128, 320], f32, name=f"ht{m}{n}", tag=f"hm{m}{n}")
                  for n in range(2)] for m in range(2)]
        for m in range(2):
            for n in range(2):
                for kt in range(4, 12):
                    if kt < 8:
                        rhs = rep10(pgT_sb[:, kt - 4, :], n)
                    else:
                        rhs = nghT_sb[:, kt - 8, n * 320:(n + 1) * 320]
                    nc.tensor.matmul(HT_ps[m][n],
                                     W1_sb[:, kt, ts(m, 128)], rhs,
                                     start=(kt == 4), stop=False)

        st2 = l2p.tile([1, 2 * D], f32)
        bn_vec(star2, g2_sb, be2_sb, st2)
        stb2 = l2p.tile([128, 2 * D], f32)
        bn_broadcast(st2, stb2)

        for bidx in range(NBLK):
            t1 = l2t.tile([128, D], f32, tag="ap1")
            nc.vector.tensor_mul(t1, u2_sb[:, bidx, :], stb2[:, 0:D])
            t2 = l2t.tile([128, D], f32, tag="ap2")
            nc.vector.tensor_add(t2, t1, stb2[:, D:2 * D])
            nc.scalar.activation(h2_sb[:, bidx, :], t2, AF.Relu)

        # =================== Readout ===================
        qs_ps = psStat.tile([SLOTS, D], f32, tag="sum")
        for bidx in range(NBLK):
            nc.tensor.matmul(qs_ps, C_sb[:, bidx, :], h2_sb[:, bidx, :],
                             start=(bidx == 0), stop=(bidx == NBLK - 1))
        qs_sb = l2p.tile([SLOTS, D], f32)
        nc.vector.tensor_copy(qs_sb, qs_ps)
        nc.sync.dma_start(out=q_in[:], in_=qs_sb)
        nc.gpsimd.collective_compute(
            "AllGather", ALU.bypass, replica_groups=RG,
            ins=[q_in.opt()], outs=[q_out.opt()])
        qg_sb = l2p.tile([NCORES * SLOTS, D], f32)
        nc.sync.dma_start(out=qg_sb, in_=q_out[:])
        qsum_ps = psStat.tile([B, D], f32, tag="ssq")
        nc.tensor.matmul(qsum_ps, Msel_sb, qg_sb, start=True, stop=True)
        qemb_sb = l2p.tile([B, D], f32)
        nc.scalar.activation(qemb_sb, qsum_ps, AF.Copy, scale=invcnt_sb)

        qT_sb = keep.tile([128, 4, B], bf)
        for j in range(4):
            qT_ps = psA.tile([128, B], f32, tag="work")
            nc.tensor.transpose(qT_ps, qemb_sb[:, ts(j, 128)], ident)
            nc.vector.tensor_copy(qT_sb[:, j, :], qT_ps)

        l2ctx.close()

        # =================== Head (bf16, feature-major) ===================
        # finish MM1 with the qemb k-tiles
        for m in range(2):
            for n in range(2):
                for kt in range(4):
                    rhs = rep10(qT_sb[:, kt, :], n)
                    nc.tensor.matmul(HT_ps[m][n],
                                     W1_sb[:, kt, ts(m, 128)], rhs,
                                     start=False, stop=(kt == 3))
        HT = hx.tile([128, 2, 640], f32, tag="hpre")
        for m in range(2):
            for n in range(2):
                nc.vector.tensor_copy(HT[:, m, n * 320:(n + 1) * 320],
                                      HT_ps[m][n])

        def head_bn_relu(pre_sb, gT, bT_, out_sb):
            """pre_sb [128, 2, 640] f32; BN over 640 rows + ReLU -> bf16 out_sb."""
            for m in range(2):
                cat = pre_sb[:, m, :]
                sums = hv.tile([128, 1], f32, tag="hsum")
                nc.vector.tensor_reduce(sums, cat, mybir.AxisListType.X,
                                        ALU.add)
                sqj = hv.tile([128, 640], f32, tag="hsq")
                ssq = hv.tile([128, 1], f32, tag="hssq")
                nc.scalar.activation(sqj, cat, AF.Square, accum_out=ssq)
                mean = hv.tile([128, 1], f32, tag="hmean")
                nc.vector.tensor_scalar_mul(mean, sums, 1.0 / 640.0)
                ex2 = hv.tile([128, 1], f32, tag="hex2")
                nc.vector.tensor_scalar_mul(ex2, ssq, 1.0 / 640.0)
                msq = hv.tile([128, 1], f32, tag="hmsq")
                nc.vector.tensor_mul(msq, mean, mean)
                var = hv.tile([128, 1], f32, tag="hvar")
                nc.vector.tensor_sub(var, ex2, msq)
                sd = hv.tile([128, 1], f32, tag="hsd")
                nc.scalar.activation(sd, var, AF.Sqrt, bias=eps128)
                rstd = hv.tile([128, 1], f32, tag="hrstd")
                nc.vector.reciprocal(rstd, sd)
                s = hv.tile([128, 1], f32, tag="hs")
                nc.vector.tensor_mul(s, rstd, gT[:, m:m + 1])
                ms = hv.tile([128, 1], f32, tag="hms")
                nc.vector.tensor_mul(ms, mean, s)
                t = hv.tile([128, 1], f32, tag="ht")
                nc.vector.tensor_sub(t, bT_[:, m:m + 1], ms)
                nc.scalar.activation(out_sb[:, m, :], cat, AF.Relu,
                                     scale=s, bias=t)

        def head_layer_mm(rhs_in, W_sb, pre_sb):
            for m in range(2):
                for n in range(2):
                    ps = psH.tile([128, 320], f32, tag=f"hm{m}{n}")
                    for kt in range(2):
                        nc.tensor.matmul(ps, W_sb[:, kt, ts(m, 128)],
                                         rhs_in[:, kt, n * 320:(n + 1) * 320],
                                         start=(kt == 0), stop=(kt == 1))
                    nc.vector.tensor_copy(pre_sb[:, m, n * 320:(n + 1) * 320],
                                          ps)

        H1h = hx.tile([128, 2, 640], bf, tag="hact")
        head_bn_relu(HT, gbT_sb, bbT_sb, H1h)

        H2p = hx.tile([128, 2, 640], f32, tag="hpre")
        head_layer_mm(H1h, W2_sb, H2p)
        H2h = hx.tile([128, 2, 640], bf, tag="hact")
        head_bn_relu(H2p, gb2T_sb, bb2T_sb, H2h)

        H3p = hx.tile([128, 2, 640], f32, tag="hpre")
        head_layer_mm(H2h, W3_sb, H3p)
        H3h = hx.tile([128, 2, 640], bf, tag="hact")
        head_bn_relu(H3p, gb3T_sb, bb3T_sb, H3h)

        pred_sb = hp.tile([128, 5], f32)
        for rr in range(5):
            pr_ps = psA.tile([128, 1], f32, tag="work")
            for kt in range(2):
                nc.tensor.matmul(pr_ps, H3h[:, kt, ts(rr, 128)],
                                 W4_sb[:, kt, :],
                                 start=(kt == 0), stop=(kt == 1))
            nc.scalar.activation(pred_sb[:, rr:rr + 1], pr_ps, AF.Sigmoid,
                                 bias=b4_sb)

        nc.sync.dma_start(
            out=bass.AP(tensor=outp.tensor, offset=outp.offset,
                        ap=[[1, 128], [128, 5]]),
            in_=pred_sb)
    nc.compile()
    return nc


# ---------------------------------------------------------------------------
# Entry point
# ---------------------------------------------------------------------------

def kernel(**inputs) -> np.ndarray:
    global LAST_EXEC_NS
    from concourse.bass_utils import run_bass_kernel_spmd

    in_maps, T_BLK, SLOTS = preprocess(**inputs)
    nc = build_nc(T_BLK, SLOTS)

    trace = bool(int(os.environ.get("GNN_TRACE", "0")))
    kw = {}
    if trace:
        kw = dict(trace=True, trace_cores=list(range(NCORES)),
                  stitch_traces=False)
    try:
        res = run_bass_kernel_spmd(nc, in_maps, core_ids=list(range(NCORES)),
                                   **kw)
    except Exception:
        if not trace:
            raise
        res = run_bass_kernel_spmd(nc, in_maps, core_ids=list(range(NCORES)))
    LAST_EXEC_NS = res.exec_time_ns
    return np.asarray(res.results[0]["outp"], np.float32)
